# revision 17
# baseline (speedup 1.0000x reference)
"""Dilated (LongNet-style) attention kernel for 8 TRN2 NeuronCores.

Strategy (head-sharded, single AllToAll):
  - Core c owns heads {c, 8+c} (slot A / slot B). Slot A heads have branch-1
    dilation offset 0, slot B offset 1 (core-uniform), so the SPMD program is
    identical across cores; branch-2 offsets (c//4, 2+c//4) are selected via
    0/1 indicator scalars supplied as data.
  - Each core computes Q^T/K^T/V^T for its 128-feature slice from the FULL
    (host-transposed) inputs, runs all 14 of its segment attentions fully
    on-chip (scores transposed: [key, query] layout so the softmax needs no
    cross-partition reduction; Z row-sums come from an appended ones column
    in V), merges the three branches by scatter-accumulation, normalizes,
    then AllToAlls the merged head-block activations so each core can run
    the output projection for its own 512-position block.
  - All matmuls run in fp32r (TF32) at full PE rate. Jobs are emitted
    interleaved with the projection position-blocks they depend on so PE/ACT
    work overlaps the input streaming.
"""

import sys

if "/opt/trn_rl_repo" not in sys.path:
    sys.path.insert(0, "/opt/trn_rl_repo")

import numpy as np

import concourse.bacc as bacc
import concourse.bass as bass
import concourse.mybir as mybir
import concourse.tile as tile
from concourse import bass_utils

F32 = mybir.dt.float32
F32R = mybir.dt.float32r
AF = mybir.ActivationFunctionType

N_CORES = 8
E, L, H, D = 1024, 4096, 16, 64
KC = 8          # contraction chunks of 128 for the projections
PB = 512        # position block
NPB = L // PB   # 8
G = 1024        # compressed segment length (same for every branch)
VBW = 65        # V_both per-chunk width (64 feats + ones col)

JOBS = [(0, 0), (0, 1), (1, 0), (0, 2), (0, 3), (1, 1), (2, 0)]


def _build(stage=6, dbg=False):
    """stage: 1=proj only, 2=+b2 copies, 3=+job b0s0, 4=+all jobs,
    5=+normalization, 6=full (a2a+outproj). dbg adds intermediate outputs."""
    nc = bacc.Bacc("TRN2", target_bir_lowering=False, debug=False,
                   num_devices=N_CORES)

    qT = nc.dram_tensor("qT", [E, L], F32R, kind="ExternalInput")
    kT = nc.dram_tensor("kT", [E, L], F32R, kind="ExternalInput")
    vT = nc.dram_tensor("vT", [E, L], F32R, kind="ExternalInput")
    wq = nc.dram_tensor("wq", [E, 128], F32R, kind="ExternalInput")
    wk = nc.dram_tensor("wk", [E, 128], F32R, kind="ExternalInput")
    wv = nc.dram_tensor("wv", [E, 128], F32R, kind="ExternalInput")
    wo = nc.dram_tensor("wo", [2, 128, 4 * E], F32R, kind="ExternalInput")
    bq = nc.dram_tensor("bq", [128, 1], F32, kind="ExternalInput")
    bk = nc.dram_tensor("bk", [128, 1], F32, kind="ExternalInput")
    bo8 = nc.dram_tensor("bo8", [128, 8], F32, kind="ExternalInput")
    ind2 = nc.dram_tensor("ind2", [2, 128], F32R, kind="ExternalInput")
    eyer = nc.dram_tensor("eyer", [128, 128], F32R, kind="ExternalInput")
    ones16 = nc.dram_tensor("ones16", [128, 16], F32R, kind="ExternalInput")
    wsel = nc.dram_tensor("wsel", [128, 2], F32, kind="ExternalInput")

    outT = nc.dram_tensor("outT", [E, PB], F32, kind="ExternalOutput")
    if dbg:
        dbg_qt = nc.dram_tensor("dbg_qt", [128, L], F32, kind="ExternalOutput")
        dbg_kt = nc.dram_tensor("dbg_kt", [128, L], F32, kind="ExternalOutput")
        dbg_vt = nc.dram_tensor("dbg_vt", [128, L], F32, kind="ExternalOutput")
        dbg_q2 = nc.dram_tensor("dbg_q2", [128, G], F32, kind="ExternalOutput")
        dbg_acc = nc.dram_tensor("dbg_acc", [128, L], F32, kind="ExternalOutput")
        dbg_zz = nc.dram_tensor("dbg_zz", [65, L], F32, kind="ExternalOutput")

    a2a_in = [nc.dram_tensor(f"a2a_in{h}", [8, 128, PB // 4], F32R)
              for h in range(4)]
    a2a_out = [nc.dram_tensor(f"a2a_out{h}", [8, 128, PB // 4], F32R)
               for h in range(4)]

    import contextlib

    def _emit(tc, ctx):
        pin = ctx.enter_context(tc.tile_pool(name="pin", bufs=2))
        persist = ctx.enter_context(tc.tile_pool(name="persist", bufs=1))
        vpool = ctx.enter_context(tc.tile_pool(name="vpool", bufs=2))
        epool = ctx.enter_context(tc.tile_pool(name="epool", bufs=4))
        opool = ctx.enter_context(tc.tile_pool(name="opool", bufs=1))
        psw = ctx.enter_context(tc.tile_pool(name="psw", bufs=2, space="PSUM"))
        pso = ctx.enter_context(tc.tile_pool(name="pso", bufs=2, space="PSUM"))

        # ---- small constants ----
        wq_sb = persist.tile([128, KC * 128], F32R, tag="wq")
        wk_sb = persist.tile([128, KC * 128], F32R, tag="wk")
        wv_sb = persist.tile([128, KC * 128], F32R, tag="wv")
        for w_d, w_t in ((wq, wq_sb), (wk, wk_sb), (wv, wv_sb)):
            wr = w_d.rearrange("(kc p) f -> kc p f", p=128)
            for kc in range(KC):
                nc.sync.dma_start(w_t[:, kc * 128:(kc + 1) * 128], wr[kc])
        bq_sb = persist.tile([128, 1], F32, tag="bq")
        bk_sb = persist.tile([128, 1], F32, tag="bk")
        bo_sb = persist.tile([128, 8], F32, tag="bo")
        ind_sb = persist.tile([2, 128], F32R, tag="ind")
        eye_sb = persist.tile([128, 128], F32R, tag="eye")
        on_sb = persist.tile([128, 16], F32R, tag="on")
        ws_sb = persist.tile([128, 2], F32, tag="ws")
        nc.sync.dma_start(bq_sb[:], bq[:])
        nc.sync.dma_start(bk_sb[:], bk[:])
        nc.sync.dma_start(bo_sb[:], bo8[:])
        nc.sync.dma_start(ind_sb[:], ind2[:])
        nc.sync.dma_start(eye_sb[:], eyer[:])
        nc.sync.dma_start(on_sb[:], ones16[:])
        nc.sync.dma_start(ws_sb[:], wsel[:])

        QT = persist.tile([128, L], F32R, tag="QT")
        KT = persist.tile([128, L], F32R, tag="KT")
        VT = persist.tile([128, L], F32R, tag="VT")
        QT2 = persist.tile([128, G], F32R, tag="QT2")
        KT2 = persist.tile([128, G], F32R, tag="KT2")
        VT2 = persist.tile([128, G], F32R, tag="VT2")
        acc = persist.tile([128, L], F32, tag="acc")
        zz = persist.tile([65, L], F32, tag="zz")

        streams = (
            ("k", kT, wk_sb, KT, bk_sb),
            ("v", vT, wv_sb, VT, None),
            ("q", qT, wq_sb, QT, bq_sb),
        )

        def proj_pb(pb):
            for name, x_d, w_t, dst, bias in streams:
                xin = pin.tile([128, KC * PB], F32R, tag="xin")
                xr = x_d.rearrange("(kc p) l -> kc p l", p=128)
                for kc in range(KC):
                    eng = (nc.sync, nc.gpsimd, nc.scalar)[kc % 3]
                    eng.dma_start(
                        xin[:, kc * PB:(kc + 1) * PB],
                        xr[kc][:, pb * PB:(pb + 1) * PB],
                    )
                pt = psw.tile([128, 1024], F32, tag="w")
                for kc in range(KC):
                    nc.tensor.matmul(
                        pt[:, 0:PB],
                        w_t[:, kc * 128:(kc + 1) * 128],
                        xin[:, kc * PB:(kc + 1) * PB],
                        start=(kc == 0), stop=(kc == KC - 1),
                    )
                dslice = dst[:, pb * PB:(pb + 1) * PB]
                if bias is not None:
                    nc.vector.tensor_scalar_add(dslice, pt[:, 0:PB], bias[:])
                else:
                    nc.vector.tensor_copy(dslice, pt[:, 0:PB])

        def b2_copies():
            # slot A picks dense offset 0 or 1, slot B picks 2 or 3, via
            # 0/1 indicators in ws_sb (core-uniform instruction stream).
            for src, dst in ((QT, QT2), (KT, KT2), (VT, VT2)):
                for slot in range(2):
                    p0, p1 = 64 * slot, 64 * slot + 64
                    o0 = 2 * slot
                    nc.vector.tensor_scalar_mul(
                        dst[p0:p1, :], src[p0:p1, o0::4], ws_sb[p0:p1, 0:1]
                    )
                    nc.vector.scalar_tensor_tensor(
                        dst[p0:p1, :], src[p0:p1, o0 + 1::4],
                        ws_sb[p0:p1, 1:2], dst[p0:p1, :],
                        mybir.AluOpType.mult, mybir.AluOpType.add,
                    )

        def kq_slice(br, seg, slot, t, lo, size):
            if br == 0:
                base = 1024 * seg + lo
                return t[slot * 64:(slot + 1) * 64, base:base + size]
            if br == 1:
                base = 2048 * seg + 2 * lo + slot
                return t[slot * 64:(slot + 1) * 64,
                         base:base + 2 * size - slot:2]
            return t[slot * 64:(slot + 1) * 64, lo:lo + size]

        def mm_ranges(jc):
            if jc <= 3:
                return [(128 * jc, 512 - 128 * jc), (512, 512)]
            return [(128 * jc, 1024 - 128 * jc)]

        def job(br, seg):
            kt_src = KT2 if br == 2 else KT
            qt_src = QT2 if br == 2 else QT
            # -- V_both prep --
            vb = vpool.tile([128, 2 * 8 * VBW], F32R, tag="vb")
            nc.vector.tensor_copy(vb[:, 64::VBW], on_sb[:])
            for jc in range(8):
                if br == 0:
                    tp = psw.tile([128, 1024], F32R, tag="w")
                    src = VT[:, 1024 * seg + 128 * jc:1024 * seg + 128 * (jc + 1)]
                    nc.tensor.transpose(tp[:, 0:128], src, eye_sb[:])
                    dst = vb[:].rearrange(
                        "p (s jj t) -> p s jj t", s=2, jj=8
                    )[:, :, jc, 0:64]
                    srcp = tp[:, 0:128].rearrange("p (s r) -> p s r", s=2)
                    nc.vector.tensor_copy(dst, srcp)
                else:
                    for slot in range(2):
                        tp = psw.tile([128, 1024], F32R, tag="w")
                        if br == 1:
                            base = 2048 * seg + 256 * jc + slot
                            src = VT[slot * 64:(slot + 1) * 64,
                                     base:base + 256 - slot:2]
                        else:
                            src = VT2[slot * 64:(slot + 1) * 64,
                                      128 * jc:128 * (jc + 1)]
                        nc.tensor.transpose(
                            tp[:, 0:64], src,
                            eye_sb[slot * 64:(slot + 1) * 64,
                                   slot * 64:(slot + 1) * 64],
                        )
                        nc.vector.tensor_copy(
                            vb[:, slot * 8 * VBW + jc * VBW:
                               slot * 8 * VBW + jc * VBW + 64],
                            tp[:, 0:64],
                        )

            o_ps_a = pso.tile([128, 1024], F32, tag="o")
            o_ps_b = pso.tile([128, 1024], F32, tag="o")
            o_ps = [o_ps_a, o_ps_b]

            for jc in range(8):
                s_ps_a = psw.tile([128, 1024], F32, tag="w")
                s_ps_b = psw.tile([128, 1024], F32, tag="w")
                s_ps = [s_ps_a, s_ps_b]
                for slot in range(2):
                    for lo, size in mm_ranges(jc):
                        lhs = kq_slice(br, seg, slot, kt_src, 128 * jc, 128)
                        rhs = kq_slice(br, seg, slot, qt_src, lo, size)
                        nc.tensor.matmul(
                            s_ps[slot][:, lo:lo + size], lhs, rhs,
                            start=True, stop=True,
                            tile_position=(slot * 64, 0),
                        )
                e_t = [None, None]
                for slot in range(2):
                    c0 = 128 * jc
                    e = epool.tile([128, 1024], F32R, tag="e")  # noqa
                    nc.scalar.activation(
                        e[:, c0:1024], s_ps[slot][:, c0:1024], AF.Exp
                    )
                    nc.gpsimd.affine_select(
                        e[:, c0:c0 + 128], e[:, c0:c0 + 128],
                        pattern=[[1, 128]],
                        compare_op=mybir.AluOpType.is_ge,
                        fill=0.0, base=0, channel_multiplier=-1,
                    )
                    e_t[slot] = e
                for slot in range(2):
                    for lo, size in mm_ranges(jc):
                        nc.tensor.matmul(
                            o_ps[slot][0:VBW, lo:lo + size],
                            vb[:, slot * 8 * VBW + jc * VBW:
                               slot * 8 * VBW + (jc + 1) * VBW],
                            e_t[slot][:, lo:lo + size],
                            start=(jc == 0), stop=(jc == 7),
                        )

            # -- merge into acc / zz (slot B copies on ACT to offload DVE) --
            for slot in range(2):
                op = o_ps[slot]
                po = slot * 64
                zr = 64 * slot
                if br == 0:
                    sl_ = slice(1024 * seg, 1024 * (seg + 1))
                    if slot == 0:
                        nc.vector.tensor_copy(acc[po:po + 64, sl_], op[0:64, :])
                        nc.vector.tensor_copy(zz[zr:zr + 1, sl_], op[64:65, :])
                    else:
                        nc.scalar.copy(acc[po:po + 64, sl_], op[0:64, :])
                        nc.scalar.copy(zz[zr:zr + 1, sl_], op[64:65, :])
                elif br == 1:
                    ac = acc[po:po + 64, 2048 * seg + slot:2048 * (seg + 1):2]
                    nc.vector.tensor_add(ac, ac, op[0:64, :])
                    zc = zz[zr:zr + 1, 2048 * seg + slot:2048 * (seg + 1):2]
                    nc.vector.tensor_add(zc, zc, op[64:65, :])
                else:
                    o0 = 2 * slot
                    for dd in range(2):
                        ac = acc[po:po + 64, o0 + dd::4]
                        nc.vector.scalar_tensor_tensor(
                            ac, op[0:64, :], ws_sb[po:po + 64, dd:dd + 1],
                            ac, mybir.AluOpType.mult, mybir.AluOpType.add,
                        )
                        zc = zz[zr:zr + 1, o0 + dd::4]
                        nc.vector.scalar_tensor_tensor(
                            zc, op[64:65, :], ws_sb[zr:zr + 1, dd:dd + 1],
                            zc, mybir.AluOpType.mult, mybir.AluOpType.add,
                        )

        # ================= emission order =================
        proj_pb(0)
        proj_pb(1)
        if dbg and stage <= 1:
            for pbx in range(2, NPB):
                proj_pb(pbx)
            nc.sync.dma_start(dbg_qt[:], QT[:].bitcast(F32))
            nc.sync.dma_start(dbg_kt[:], KT[:].bitcast(F32))
            nc.sync.dma_start(dbg_vt[:], VT[:].bitcast(F32))
            return
        if stage >= 3:
            job(0, 0)
        proj_pb(2)
        proj_pb(3)
        if stage >= 4:
            job(0, 1)
            job(1, 0)
        proj_pb(4)
        proj_pb(5)
        if stage >= 4:
            job(0, 2)
        proj_pb(6)
        proj_pb(7)
        b2_copies()
        if stage >= 4:
            job(0, 3)
            job(1, 1)
            job(2, 0)

        if dbg:
            nc.sync.dma_start(dbg_qt[:], QT[:].bitcast(F32))
            nc.sync.dma_start(dbg_kt[:], KT[:].bitcast(F32))
            nc.sync.dma_start(dbg_vt[:], VT[:].bitcast(F32))
            nc.sync.dma_start(dbg_q2[:], QT2[:].bitcast(F32))
            if stage >= 3:
                nc.sync.dma_start(dbg_acc[:], acc[:])
                nc.sync.dma_start(dbg_zz[:], zz[:])
        if stage <= 4:
            return

        # ---- normalization (reciprocal reshaped to use all 128 lanes) ----
        zw = persist.tile([128, 64], F32R, tag="zw")
        for i, zr in enumerate((0, 64)):
            nc.sync.dma_start(
                zw[:, 32 * i:32 * i + 32].bitcast(F32), zz[zr:zr + 1, :]
            )
        with nc.allow_low_precision(reason="tf32 norm"):
            nc.vector.reciprocal(zw[:], zw[:])
        for pb in range(NPB):
            rzp = opool.tile([2, PB], F32R, tag="rzp")
            nc.sync.dma_start(rzp[0:1, :], zw[16 * pb:16 * pb + 16, 0:32])
            nc.sync.dma_start(rzp[1:2, :], zw[16 * pb:16 * pb + 16, 32:64])
            rb = psw.tile([128, 1024], F32, tag="w")
            nc.tensor.matmul(
                rb[:, 0:PB], ind_sb[:], rzp[:], start=True, stop=True,
            )
            aslice = acc[:, pb * PB:(pb + 1) * PB]
            nc.vector.tensor_mul(aslice, aslice, rb[:, 0:PB])
        if dbg and stage == 5:
            nc.sync.dma_start(dbg_acc[:], acc[:])
        if stage <= 5:
            return

        # ---- chunked AllToAll + output projection pipeline ----
        # split the position block into NCH chunks; a2a chunk h then overlaps
        # the output projection of chunk h-1.
        wo_sb_0 = pin.tile([128, 4 * E], F32R, tag="xin")
        wo_sb_1 = pin.tile([128, 4 * E], F32R, tag="xin")
        wo_sb = [wo_sb_0, wo_sb_1]
        for g in range(2):
            nc.sync.dma_start(wo_sb[g][:], wo[g])
        NCH = 4
        CW = PB // NCH  # chunk width within each 512 block
        mg = persist.tile([128, 8 * PB], F32R, tag="mg")
        for h in range(NCH):
            for r in range(8):
                nc.sync.dma_start(
                    a2a_in[h][r],
                    acc[:, PB * r + CW * h:PB * r + CW * (h + 1)].bitcast(F32R),
                )
            nc.gpsimd.collective_compute(
                "AllToAll", mybir.AluOpType.bypass,
                replica_groups=[list(range(8))],
                ins=[a2a_in[h][:]], outs=[a2a_out[h][:]],
            )
            for s in range(8):
                nc.sync.dma_start(
                    mg[:, s * PB + CW * h:s * PB + CW * (h + 1)], a2a_out[h][s]
                )
            for ob in range(8):
                pt = psw.tile([128, 1024], F32, tag="w")
                for ec in range(KC):
                    w_t = wo_sb[ec // 4]
                    lhs = w_t[:, (ec % 4) * E + ob * 128:
                              (ec % 4) * E + (ob + 1) * 128]
                    nc.tensor.matmul(
                        pt[:, 0:CW], lhs,
                        mg[:, ec * PB + CW * h:ec * PB + CW * (h + 1)],
                        start=(ec == 0), stop=(ec == KC - 1),
                    )
                osb = opool.tile([128, CW], F32, tag="osb")
                nc.vector.tensor_scalar_add(
                    osb[:], pt[:, 0:CW], bo_sb[:, ob:ob + 1]
                )
                nc.sync.dma_start(
                    outT[ob * 128:(ob + 1) * 128, CW * h:CW * (h + 1)], osb[:]
                )

    with tile.TileContext(nc) as tc, contextlib.ExitStack() as ctx:
        _emit(tc, ctx)

    nc.compile()
    return nc


_NC_CACHE = {}


def _get_nc(stage=6, dbg=False):
    key = (stage, dbg)
    if key not in _NC_CACHE:
        _NC_CACHE[key] = _build(stage, dbg)
    return _NC_CACHE[key]


def _prep_inputs(query, key, value, Wq, bq, Wk, bk, Wv, bv, Wo, bo):
    """Host-side sharding/layout prep. Returns in_maps for the 8 cores."""
    qT = np.ascontiguousarray(query[0].T)  # (E, L)
    kT = np.ascontiguousarray(key[0].T)
    vT = np.ascontiguousarray(value[0].T)

    WqT = np.ascontiguousarray(Wq.T) * np.float32(0.125)
    WkT = np.ascontiguousarray(Wk.T)
    WvT = np.ascontiguousarray(Wv.T)

    # permuted Wo.T rows to match a2a feature order
    perm = np.concatenate(
        [np.r_[64 * s:64 * s + 64, 512 + 64 * s:512 + 64 * s + 64]
         for s in range(8)]
    )
    WoT = np.ascontiguousarray(Wo.T)[perm]  # (E e', E o)
    wo_pack = np.zeros((2, 128, 4 * E), np.float32)
    for ec in range(8):
        wo_pack[ec // 4, :, (ec % 4) * E:(ec % 4 + 1) * E] = \
            WoT[ec * 128:(ec + 1) * 128]

    bo_eff = (bo + bv @ Wo.T).astype(np.float32)
    bo8 = bo_eff.reshape(8, 128).T.copy()  # [p, ob]

    # per-core offset indicators: slot A offset = c//4 in {0,1} on rows 0-63,
    # slot B offset = 2 + c//4 (encoded as its low bit) on rows 64-127.
    WS = np.zeros((8, 128, 2), np.float32)
    for c in range(8):
        d = c // 4
        WS[c, 0:64, d] = 1.0
        WS[c, 64:128, d] = 1.0

    IND = np.zeros((2, 128), np.float32)
    IND[0, 0:64] = 1.0
    IND[1, 64:128] = 1.0
    EYE = np.eye(128, dtype=np.float32)
    ONES16 = np.ones((128, 16), np.float32)

    in_maps = []
    for c in range(8):
        fa = np.r_[64 * c:64 * c + 64]
        fb = np.r_[512 + 64 * c:512 + 64 * c + 64]
        sel = np.concatenate([fa, fb])
        in_maps.append({
            "qT": qT, "kT": kT, "vT": vT,
            "wq": np.ascontiguousarray(WqT[:, sel]),
            "wk": np.ascontiguousarray(WkT[:, sel]),
            "wv": np.ascontiguousarray(WvT[:, sel]),
            "wo": wo_pack,
            "bq": (bq[sel] * np.float32(0.125)).reshape(128, 1).astype(np.float32),
            "bk": bk[sel].reshape(128, 1).astype(np.float32),
            "bo8": bo8,
            "ind2": IND, "eyer": EYE, "ones16": ONES16,
            "wsel": WS[c],
        })
    return in_maps


def kernel(query, key, value, Wq, bq, Wk, bk, Wv, bv, Wo, bo,
           _trace=False, _result_holder=None, _stage=6, _dbg=False):
    args = [np.asarray(a, np.float32) for a in
            (query, key, value, Wq, bq, Wk, bk, Wv, bv, Wo, bo)]
    nc = _get_nc(_stage, _dbg)
    in_maps = _prep_inputs(*args)
    res = bass_utils.run_bass_kernel_spmd(
        nc, in_maps, core_ids=list(range(N_CORES)), trace=_trace
    )
    if _result_holder is not None:
        _result_holder.append(res)
    outT = np.zeros((E, L), np.float32)
    for c in range(N_CORES):
        outT[:, PB * c:PB * (c + 1)] = res.results[c]["outT"]
    return np.ascontiguousarray(outT.T).reshape(1, L, E)


# revision 20
# speedup vs baseline: 1.0281x; 1.0281x over previous
"""Dilated (LongNet-style) attention kernel for 8 TRN2 NeuronCores.

Strategy (head-sharded, single AllToAll):
  - Core c owns heads {c, 8+c} (slot A / slot B). Slot A heads have branch-1
    dilation offset 0, slot B offset 1 (core-uniform), so the SPMD program is
    identical across cores; branch-2 offsets (c//4, 2+c//4) are selected via
    0/1 indicator scalars supplied as data.
  - Each core computes Q^T/K^T/V^T for its 128-feature slice from the FULL
    (host-transposed) inputs, runs all 14 of its segment attentions fully
    on-chip (scores transposed: [key, query] layout so the softmax needs no
    cross-partition reduction; Z row-sums come from an appended ones column
    in V), merges the three branches by scatter-accumulation, normalizes,
    then AllToAlls the merged head-block activations so each core can run
    the output projection for its own 512-position block.
  - All matmuls run in fp32r (TF32) at full PE rate. Jobs are emitted
    interleaved with the projection position-blocks they depend on so PE/ACT
    work overlaps the input streaming.
"""

import sys

if "/opt/trn_rl_repo" not in sys.path:
    sys.path.insert(0, "/opt/trn_rl_repo")

import numpy as np

import concourse.bacc as bacc
import concourse.bass as bass
import concourse.mybir as mybir
import concourse.tile as tile
from concourse import bass_utils

F32 = mybir.dt.float32
F32R = mybir.dt.float32r
AF = mybir.ActivationFunctionType

N_CORES = 8
E, L, H, D = 1024, 4096, 16, 64
KC = 8          # contraction chunks of 128 for the projections
PB = 512        # position block
NPB = L // PB   # 8
G = 1024        # compressed segment length (same for every branch)
VBW = 65        # V_both per-chunk width (64 feats + ones col)

JOBS = [(0, 0), (0, 1), (1, 0), (0, 2), (0, 3), (1, 1), (2, 0)]


def _build(stage=6, dbg=False):
    """stage: 1=proj only, 2=+b2 copies, 3=+job b0s0, 4=+all jobs,
    5=+normalization, 6=full (a2a+outproj). dbg adds intermediate outputs."""
    nc = bacc.Bacc("TRN2", target_bir_lowering=False, debug=False,
                   num_devices=N_CORES)

    qT = nc.dram_tensor("qT", [E, L], F32R, kind="ExternalInput")
    kT = nc.dram_tensor("kT", [E, L], F32R, kind="ExternalInput")
    vT = nc.dram_tensor("vT", [E, L], F32R, kind="ExternalInput")
    wq = nc.dram_tensor("wq", [E, 128], F32R, kind="ExternalInput")
    wk = nc.dram_tensor("wk", [E, 128], F32R, kind="ExternalInput")
    wv = nc.dram_tensor("wv", [E, 128], F32R, kind="ExternalInput")
    wo = nc.dram_tensor("wo", [2, 128, 4 * E], F32R, kind="ExternalInput")
    bq = nc.dram_tensor("bq", [128, 1], F32, kind="ExternalInput")
    bk = nc.dram_tensor("bk", [128, 1], F32, kind="ExternalInput")
    bo8 = nc.dram_tensor("bo8", [128, 8], F32, kind="ExternalInput")
    ind2 = nc.dram_tensor("ind2", [2, 128], F32R, kind="ExternalInput")
    eyer = nc.dram_tensor("eyer", [128, 128], F32R, kind="ExternalInput")
    ones16 = nc.dram_tensor("ones16", [128, 16], F32R, kind="ExternalInput")
    wsel = nc.dram_tensor("wsel", [128, 2], F32, kind="ExternalInput")

    outT = nc.dram_tensor("outT", [E, PB], F32, kind="ExternalOutput")
    if dbg:
        dbg_qt = nc.dram_tensor("dbg_qt", [128, L], F32, kind="ExternalOutput")
        dbg_kt = nc.dram_tensor("dbg_kt", [128, L], F32, kind="ExternalOutput")
        dbg_vt = nc.dram_tensor("dbg_vt", [128, L], F32, kind="ExternalOutput")
        dbg_q2 = nc.dram_tensor("dbg_q2", [128, G], F32, kind="ExternalOutput")
        dbg_acc = nc.dram_tensor("dbg_acc", [128, L], F32, kind="ExternalOutput")
        dbg_zz = nc.dram_tensor("dbg_zz", [65, L], F32, kind="ExternalOutput")

    a2a_warm_in = nc.dram_tensor("a2a_warm_in", [8, 1, 64], F32R)
    a2a_warm_out = nc.dram_tensor("a2a_warm_out", [8, 1, 64], F32R)
    a2a_in = [nc.dram_tensor(f"a2a_in{h}", [8, 128, PB // 4], F32R)
              for h in range(4)]
    a2a_out = [nc.dram_tensor(f"a2a_out{h}", [8, 128, PB // 4], F32R)
               for h in range(4)]

    import contextlib

    def _emit(tc, ctx):
        pin = ctx.enter_context(tc.tile_pool(name="pin", bufs=2))
        persist = ctx.enter_context(tc.tile_pool(name="persist", bufs=1))
        vpool = ctx.enter_context(tc.tile_pool(name="vpool", bufs=2))
        epool = ctx.enter_context(tc.tile_pool(name="epool", bufs=4))
        opool = ctx.enter_context(tc.tile_pool(name="opool", bufs=1))
        psw = ctx.enter_context(tc.tile_pool(name="psw", bufs=2, space="PSUM"))
        pso = ctx.enter_context(tc.tile_pool(name="pso", bufs=2, space="PSUM"))

        # ---- small constants ----
        wq_sb = persist.tile([128, KC * 128], F32R, tag="wq")
        wk_sb = persist.tile([128, KC * 128], F32R, tag="wk")
        wv_sb = persist.tile([128, KC * 128], F32R, tag="wv")
        for w_d, w_t in ((wq, wq_sb), (wk, wk_sb), (wv, wv_sb)):
            wr = w_d.rearrange("(kc p) f -> kc p f", p=128)
            for kc in range(KC):
                nc.sync.dma_start(w_t[:, kc * 128:(kc + 1) * 128], wr[kc])
        bq_sb = persist.tile([128, 1], F32, tag="bq")
        bk_sb = persist.tile([128, 1], F32, tag="bk")
        bo_sb = persist.tile([128, 8], F32, tag="bo")
        ind_sb = persist.tile([2, 128], F32R, tag="ind")
        eye_sb = persist.tile([128, 128], F32R, tag="eye")
        on_sb = persist.tile([128, 16], F32R, tag="on")
        ws_sb = persist.tile([128, 2], F32, tag="ws")
        nc.sync.dma_start(bq_sb[:], bq[:])
        nc.sync.dma_start(bk_sb[:], bk[:])
        nc.sync.dma_start(bo_sb[:], bo8[:])
        nc.sync.dma_start(ind_sb[:], ind2[:])
        nc.sync.dma_start(eye_sb[:], eyer[:])
        nc.sync.dma_start(on_sb[:], ones16[:])
        nc.sync.dma_start(ws_sb[:], wsel[:])

        QT = persist.tile([128, L], F32R, tag="QT")
        KT = persist.tile([128, L], F32R, tag="KT")
        VT = persist.tile([128, L], F32R, tag="VT")
        QT2 = persist.tile([128, G], F32R, tag="QT2")
        KT2 = persist.tile([128, G], F32R, tag="KT2")
        VT2 = persist.tile([128, G], F32R, tag="VT2")
        acc = persist.tile([128, L], F32, tag="acc")
        zz = persist.tile([65, L], F32, tag="zz")

        streams = (
            ("k", kT, wk_sb, KT, bk_sb),
            ("v", vT, wv_sb, VT, None),
            ("q", qT, wq_sb, QT, bq_sb),
        )

        def proj_pb(pb):
            for name, x_d, w_t, dst, bias in streams:
                xin = pin.tile([128, KC * PB], F32R, tag="xin")
                xr = x_d.rearrange("(kc p) l -> kc p l", p=128)
                for kc in range(KC):
                    eng = (nc.sync, nc.gpsimd, nc.scalar)[kc % 3]
                    eng.dma_start(
                        xin[:, kc * PB:(kc + 1) * PB],
                        xr[kc][:, pb * PB:(pb + 1) * PB],
                    )
                pt = psw.tile([128, 1024], F32, tag="w")
                for kc in range(KC):
                    nc.tensor.matmul(
                        pt[:, 0:PB],
                        w_t[:, kc * 128:(kc + 1) * 128],
                        xin[:, kc * PB:(kc + 1) * PB],
                        start=(kc == 0), stop=(kc == KC - 1),
                    )
                dslice = dst[:, pb * PB:(pb + 1) * PB]
                if bias is not None:
                    nc.vector.tensor_scalar_add(dslice, pt[:, 0:PB], bias[:])
                else:
                    nc.vector.tensor_copy(dslice, pt[:, 0:PB])

        def b2_copies():
            # slot A picks dense offset 0 or 1, slot B picks 2 or 3, via
            # 0/1 indicators in ws_sb (core-uniform instruction stream).
            for src, dst in ((QT, QT2), (KT, KT2), (VT, VT2)):
                for slot in range(2):
                    p0, p1 = 64 * slot, 64 * slot + 64
                    o0 = 2 * slot
                    nc.vector.tensor_scalar_mul(
                        dst[p0:p1, :], src[p0:p1, o0::4], ws_sb[p0:p1, 0:1]
                    )
                    nc.vector.scalar_tensor_tensor(
                        dst[p0:p1, :], src[p0:p1, o0 + 1::4],
                        ws_sb[p0:p1, 1:2], dst[p0:p1, :],
                        mybir.AluOpType.mult, mybir.AluOpType.add,
                    )

        def kq_slice(br, seg, slot, t, lo, size):
            if br == 0:
                base = 1024 * seg + lo
                return t[slot * 64:(slot + 1) * 64, base:base + size]
            if br == 1:
                base = 2048 * seg + 2 * lo + slot
                return t[slot * 64:(slot + 1) * 64,
                         base:base + 2 * size - slot:2]
            return t[slot * 64:(slot + 1) * 64, lo:lo + size]

        def mm_ranges(jc):
            if jc <= 3:
                return [(128 * jc, 512 - 128 * jc), (512, 512)]
            return [(128 * jc, 1024 - 128 * jc)]

        def job(br, seg):
            kt_src = KT2 if br == 2 else KT
            qt_src = QT2 if br == 2 else QT
            # -- V_both prep --
            vb = vpool.tile([128, 2 * 8 * VBW], F32R, tag="vb")
            nc.vector.tensor_copy(vb[:, 64::VBW], on_sb[:])
            for jc in range(8):
                if br == 0:
                    tp = psw.tile([128, 1024], F32R, tag="w")
                    src = VT[:, 1024 * seg + 128 * jc:1024 * seg + 128 * (jc + 1)]
                    nc.tensor.transpose(tp[:, 0:128], src, eye_sb[:])
                    dst = vb[:].rearrange(
                        "p (s jj t) -> p s jj t", s=2, jj=8
                    )[:, :, jc, 0:64]
                    srcp = tp[:, 0:128].rearrange("p (s r) -> p s r", s=2)
                    nc.vector.tensor_copy(dst, srcp)
                else:
                    for slot in range(2):
                        tp = psw.tile([128, 1024], F32R, tag="w")
                        if br == 1:
                            base = 2048 * seg + 256 * jc + slot
                            src = VT[slot * 64:(slot + 1) * 64,
                                     base:base + 256 - slot:2]
                        else:
                            src = VT2[slot * 64:(slot + 1) * 64,
                                      128 * jc:128 * (jc + 1)]
                        nc.tensor.transpose(
                            tp[:, 0:64], src,
                            eye_sb[slot * 64:(slot + 1) * 64,
                                   slot * 64:(slot + 1) * 64],
                        )
                        nc.vector.tensor_copy(
                            vb[:, slot * 8 * VBW + jc * VBW:
                               slot * 8 * VBW + jc * VBW + 64],
                            tp[:, 0:64],
                        )

            o_ps_a = pso.tile([128, 1024], F32, tag="o")
            o_ps_b = pso.tile([128, 1024], F32, tag="o")
            o_ps = [o_ps_a, o_ps_b]

            for jc in range(8):
                s_ps_a = psw.tile([128, 1024], F32, tag="w")
                s_ps_b = psw.tile([128, 1024], F32, tag="w")
                s_ps = [s_ps_a, s_ps_b]
                for slot in range(2):
                    for lo, size in mm_ranges(jc):
                        lhs = kq_slice(br, seg, slot, kt_src, 128 * jc, 128)
                        rhs = kq_slice(br, seg, slot, qt_src, lo, size)
                        nc.tensor.matmul(
                            s_ps[slot][:, lo:lo + size], lhs, rhs,
                            start=True, stop=True,
                            tile_position=(slot * 64, 0),
                        )
                e_t = [None, None]
                for slot in range(2):
                    c0 = 128 * jc
                    e = epool.tile([128, 1024], F32R, tag="e")  # noqa
                    nc.scalar.activation(
                        e[:, c0:1024], s_ps[slot][:, c0:1024], AF.Exp
                    )
                    nc.gpsimd.affine_select(
                        e[:, c0:c0 + 128], e[:, c0:c0 + 128],
                        pattern=[[1, 128]],
                        compare_op=mybir.AluOpType.is_ge,
                        fill=0.0, base=0, channel_multiplier=-1,
                    )
                    e_t[slot] = e
                for slot in range(2):
                    for lo, size in mm_ranges(jc):
                        nc.tensor.matmul(
                            o_ps[slot][0:VBW, lo:lo + size],
                            vb[:, slot * 8 * VBW + jc * VBW:
                               slot * 8 * VBW + (jc + 1) * VBW],
                            e_t[slot][:, lo:lo + size],
                            start=(jc == 0), stop=(jc == 7),
                        )

            # -- merge into acc / zz (slot B copies on ACT to offload DVE) --
            for slot in range(2):
                op = o_ps[slot]
                po = slot * 64
                zr = 64 * slot
                if br == 0:
                    sl_ = slice(1024 * seg, 1024 * (seg + 1))
                    if slot == 0:
                        nc.vector.tensor_copy(acc[po:po + 64, sl_], op[0:64, :])
                        nc.vector.tensor_copy(zz[zr:zr + 1, sl_], op[64:65, :])
                    else:
                        nc.scalar.copy(acc[po:po + 64, sl_], op[0:64, :])
                        nc.scalar.copy(zz[zr:zr + 1, sl_], op[64:65, :])
                elif br == 1:
                    ac = acc[po:po + 64, 2048 * seg + slot:2048 * (seg + 1):2]
                    nc.vector.tensor_add(ac, ac, op[0:64, :])
                    zc = zz[zr:zr + 1, 2048 * seg + slot:2048 * (seg + 1):2]
                    nc.vector.tensor_add(zc, zc, op[64:65, :])
                else:
                    o0 = 2 * slot
                    for dd in range(2):
                        ac = acc[po:po + 64, o0 + dd::4]
                        nc.vector.scalar_tensor_tensor(
                            ac, op[0:64, :], ws_sb[po:po + 64, dd:dd + 1],
                            ac, mybir.AluOpType.mult, mybir.AluOpType.add,
                        )
                        zc = zz[zr:zr + 1, o0 + dd::4]
                        nc.vector.scalar_tensor_tensor(
                            zc, op[64:65, :], ws_sb[zr:zr + 1, dd:dd + 1],
                            zc, mybir.AluOpType.mult, mybir.AluOpType.add,
                        )

        # ================= emission order =================
        # tiny dummy collective first: absorbs the ~30us first-collective
        # setup cost while the projections stream.
        if stage >= 6:
            for rr in range(8):
                nc.sync.dma_start(a2a_warm_in[rr][0:1, 0:16], on_sb[0:1, 0:16])
            nc.gpsimd.collective_compute(
                "AllToAll", mybir.AluOpType.bypass,
                replica_groups=[list(range(8))],
                ins=[a2a_warm_in[:]], outs=[a2a_warm_out[:]],
            )
        proj_pb(0)
        proj_pb(1)
        if dbg and stage <= 1:
            for pbx in range(2, NPB):
                proj_pb(pbx)
            nc.sync.dma_start(dbg_qt[:], QT[:].bitcast(F32))
            nc.sync.dma_start(dbg_kt[:], KT[:].bitcast(F32))
            nc.sync.dma_start(dbg_vt[:], VT[:].bitcast(F32))
            return
        if stage >= 3:
            job(0, 0)
        proj_pb(2)
        proj_pb(3)
        if stage >= 4:
            job(0, 1)
            job(1, 0)
        proj_pb(4)
        proj_pb(5)
        if stage >= 4:
            job(0, 2)
        proj_pb(6)
        proj_pb(7)
        if stage >= 4:
            job(0, 3)
        b2_copies()
        if stage >= 4:
            job(2, 0)
            job(1, 1)

        if dbg:
            nc.sync.dma_start(dbg_qt[:], QT[:].bitcast(F32))
            nc.sync.dma_start(dbg_kt[:], KT[:].bitcast(F32))
            nc.sync.dma_start(dbg_vt[:], VT[:].bitcast(F32))
            nc.sync.dma_start(dbg_q2[:], QT2[:].bitcast(F32))
            if stage >= 3:
                nc.sync.dma_start(dbg_acc[:], acc[:])
                nc.sync.dma_start(dbg_zz[:], zz[:])
        if stage <= 4:
            return

        # ---- normalization (reciprocal reshaped to use all 128 lanes) ----
        zw = persist.tile([128, 64], F32R, tag="zw")
        for i, zr in enumerate((0, 64)):
            nc.sync.dma_start(
                zw[:, 32 * i:32 * i + 32].bitcast(F32), zz[zr:zr + 1, :]
            )
        with nc.allow_low_precision(reason="tf32 norm"):
            nc.vector.reciprocal(zw[:], zw[:])
        for pb in range(NPB):
            rzp = opool.tile([2, PB], F32R, tag="rzp")
            nc.sync.dma_start(rzp[0:1, :], zw[16 * pb:16 * pb + 16, 0:32])
            nc.sync.dma_start(rzp[1:2, :], zw[16 * pb:16 * pb + 16, 32:64])
            rb = psw.tile([128, 1024], F32, tag="w")
            nc.tensor.matmul(
                rb[:, 0:PB], ind_sb[:], rzp[:], start=True, stop=True,
            )
            aslice = acc[:, pb * PB:(pb + 1) * PB]
            nc.vector.tensor_mul(aslice, aslice, rb[:, 0:PB])
        if dbg and stage == 5:
            nc.sync.dma_start(dbg_acc[:], acc[:])
        if stage <= 5:
            return

        # ---- chunked AllToAll + output projection pipeline ----
        # split the position block into NCH chunks; a2a chunk h then overlaps
        # the output projection of chunk h-1.
        wo_sb_0 = pin.tile([128, 4 * E], F32R, tag="xin")
        wo_sb_1 = pin.tile([128, 4 * E], F32R, tag="xin")
        wo_sb = [wo_sb_0, wo_sb_1]
        for g in range(2):
            nc.sync.dma_start(wo_sb[g][:], wo[g])
        NCH = 4
        CW = PB // NCH  # chunk width within each 512 block
        mg = persist.tile([128, 8 * PB], F32R, tag="mg")
        for h in range(NCH):
            for r in range(8):
                nc.sync.dma_start(
                    a2a_in[h][r],
                    acc[:, PB * r + CW * h:PB * r + CW * (h + 1)].bitcast(F32R),
                )
            nc.gpsimd.collective_compute(
                "AllToAll", mybir.AluOpType.bypass,
                replica_groups=[list(range(8))],
                ins=[a2a_in[h][:]], outs=[a2a_out[h][:]],
            )
            for s in range(8):
                nc.sync.dma_start(
                    mg[:, s * PB + CW * h:s * PB + CW * (h + 1)], a2a_out[h][s]
                )
            for ob in range(8):
                pt = psw.tile([128, 1024], F32, tag="w")
                for ec in range(KC):
                    w_t = wo_sb[ec // 4]
                    lhs = w_t[:, (ec % 4) * E + ob * 128:
                              (ec % 4) * E + (ob + 1) * 128]
                    nc.tensor.matmul(
                        pt[:, 0:CW], lhs,
                        mg[:, ec * PB + CW * h:ec * PB + CW * (h + 1)],
                        start=(ec == 0), stop=(ec == KC - 1),
                    )
                osb = opool.tile([128, CW], F32, tag="osb")
                nc.vector.tensor_scalar_add(
                    osb[:], pt[:, 0:CW], bo_sb[:, ob:ob + 1]
                )
                nc.sync.dma_start(
                    outT[ob * 128:(ob + 1) * 128, CW * h:CW * (h + 1)], osb[:]
                )

    with tile.TileContext(nc) as tc, contextlib.ExitStack() as ctx:
        _emit(tc, ctx)

    nc.compile()
    return nc


_NC_CACHE = {}


def _get_nc(stage=6, dbg=False):
    key = (stage, dbg)
    if key not in _NC_CACHE:
        _NC_CACHE[key] = _build(stage, dbg)
    return _NC_CACHE[key]


def _prep_inputs(query, key, value, Wq, bq, Wk, bk, Wv, bv, Wo, bo):
    """Host-side sharding/layout prep. Returns in_maps for the 8 cores."""
    qT = np.ascontiguousarray(query[0].T)  # (E, L)
    kT = np.ascontiguousarray(key[0].T)
    vT = np.ascontiguousarray(value[0].T)

    WqT = np.ascontiguousarray(Wq.T) * np.float32(0.125)
    WkT = np.ascontiguousarray(Wk.T)
    WvT = np.ascontiguousarray(Wv.T)

    # permuted Wo.T rows to match a2a feature order
    perm = np.concatenate(
        [np.r_[64 * s:64 * s + 64, 512 + 64 * s:512 + 64 * s + 64]
         for s in range(8)]
    )
    WoT = np.ascontiguousarray(Wo.T)[perm]  # (E e', E o)
    wo_pack = np.zeros((2, 128, 4 * E), np.float32)
    for ec in range(8):
        wo_pack[ec // 4, :, (ec % 4) * E:(ec % 4 + 1) * E] = \
            WoT[ec * 128:(ec + 1) * 128]

    bo_eff = (bo + bv @ Wo.T).astype(np.float32)
    bo8 = bo_eff.reshape(8, 128).T.copy()  # [p, ob]

    # per-core offset indicators: slot A offset = c//4 in {0,1} on rows 0-63,
    # slot B offset = 2 + c//4 (encoded as its low bit) on rows 64-127.
    WS = np.zeros((8, 128, 2), np.float32)
    for c in range(8):
        d = c // 4
        WS[c, 0:64, d] = 1.0
        WS[c, 64:128, d] = 1.0

    IND = np.zeros((2, 128), np.float32)
    IND[0, 0:64] = 1.0
    IND[1, 64:128] = 1.0
    EYE = np.eye(128, dtype=np.float32)
    ONES16 = np.ones((128, 16), np.float32)

    in_maps = []
    for c in range(8):
        fa = np.r_[64 * c:64 * c + 64]
        fb = np.r_[512 + 64 * c:512 + 64 * c + 64]
        sel = np.concatenate([fa, fb])
        in_maps.append({
            "qT": qT, "kT": kT, "vT": vT,
            "wq": np.ascontiguousarray(WqT[:, sel]),
            "wk": np.ascontiguousarray(WkT[:, sel]),
            "wv": np.ascontiguousarray(WvT[:, sel]),
            "wo": wo_pack,
            "bq": (bq[sel] * np.float32(0.125)).reshape(128, 1).astype(np.float32),
            "bk": bk[sel].reshape(128, 1).astype(np.float32),
            "bo8": bo8,
            "ind2": IND, "eyer": EYE, "ones16": ONES16,
            "wsel": WS[c],
        })
    return in_maps


def kernel(query, key, value, Wq, bq, Wk, bk, Wv, bv, Wo, bo,
           _trace=False, _result_holder=None, _stage=6, _dbg=False):
    args = [np.asarray(a, np.float32) for a in
            (query, key, value, Wq, bq, Wk, bk, Wv, bv, Wo, bo)]
    nc = _get_nc(_stage, _dbg)
    in_maps = _prep_inputs(*args)
    res = bass_utils.run_bass_kernel_spmd(
        nc, in_maps, core_ids=list(range(N_CORES)), trace=_trace
    )
    if _result_holder is not None:
        _result_holder.append(res)
    outT = np.zeros((E, L), np.float32)
    for c in range(N_CORES):
        outT[:, PB * c:PB * (c + 1)] = res.results[c]["outT"]
    return np.ascontiguousarray(outT.T).reshape(1, L, E)


# revision 22
# speedup vs baseline: 1.1341x; 1.1031x over previous
"""Dilated (LongNet-style) attention kernel for 8 TRN2 NeuronCores.

Strategy (head-sharded, single AllToAll):
  - Core c owns heads {c, 8+c} (slot A / slot B). Slot A heads have branch-1
    dilation offset 0, slot B offset 1 (core-uniform), so the SPMD program is
    identical across cores; branch-2 offsets (c//4, 2+c//4) are selected via
    0/1 indicator scalars supplied as data.
  - Each core computes Q^T/K^T/V^T for its 128-feature slice from the FULL
    (host-transposed) inputs, runs all 14 of its segment attentions fully
    on-chip (scores transposed: [key, query] layout so the softmax needs no
    cross-partition reduction; Z row-sums come from an appended ones column
    in V), merges the three branches by scatter-accumulation, normalizes,
    then AllToAlls the merged head-block activations so each core can run
    the output projection for its own 512-position block.
  - All matmuls run in fp32r (TF32) at full PE rate. Jobs are emitted
    interleaved with the projection position-blocks they depend on so PE/ACT
    work overlaps the input streaming.
"""

import sys

if "/opt/trn_rl_repo" not in sys.path:
    sys.path.insert(0, "/opt/trn_rl_repo")

import numpy as np

import concourse.bacc as bacc
import concourse.bass as bass
import concourse.mybir as mybir
import concourse.tile as tile
from concourse import bass_utils

F32 = mybir.dt.float32
F32R = mybir.dt.float32r
AF = mybir.ActivationFunctionType

N_CORES = 8
E, L, H, D = 1024, 4096, 16, 64
KC = 8          # contraction chunks of 128 for the projections
PB = 512        # position block
NPB = L // PB   # 8
G = 1024        # compressed segment length (same for every branch)
VBW = 65        # V_both per-chunk width (64 feats + ones col)

JOBS = [(0, 0), (0, 1), (1, 0), (0, 2), (0, 3), (1, 1), (2, 0)]


def _build(stage=6, dbg=False):
    """stage: 1=proj only, 2=+b2 copies, 3=+job b0s0, 4=+all jobs,
    5=+normalization, 6=full (a2a+outproj). dbg adds intermediate outputs."""
    nc = bacc.Bacc("TRN2", target_bir_lowering=False, debug=False,
                   num_devices=N_CORES)

    qT = nc.dram_tensor("qT", [E, L], F32R, kind="ExternalInput")
    kT = nc.dram_tensor("kT", [E, L], F32R, kind="ExternalInput")
    vT = nc.dram_tensor("vT", [E, L], F32R, kind="ExternalInput")
    wq = nc.dram_tensor("wq", [E, 128], F32R, kind="ExternalInput")
    wk = nc.dram_tensor("wk", [E, 128], F32R, kind="ExternalInput")
    wv = nc.dram_tensor("wv", [E, 128], F32R, kind="ExternalInput")
    wo = nc.dram_tensor("wo", [2, 128, 4 * E], F32R, kind="ExternalInput")
    bq = nc.dram_tensor("bq", [128, 1], F32, kind="ExternalInput")
    bk = nc.dram_tensor("bk", [128, 1], F32, kind="ExternalInput")
    bo8 = nc.dram_tensor("bo8", [128, 8], F32, kind="ExternalInput")
    ind2 = nc.dram_tensor("ind2", [2, 128], F32R, kind="ExternalInput")
    eyer = nc.dram_tensor("eyer", [128, 128], F32R, kind="ExternalInput")
    ones16 = nc.dram_tensor("ones16", [128, 16], F32R, kind="ExternalInput")
    wsel = nc.dram_tensor("wsel", [128, 2], F32, kind="ExternalInput")

    outT = nc.dram_tensor("outT", [E, PB], F32, kind="ExternalOutput")
    if dbg:
        dbg_qt = nc.dram_tensor("dbg_qt", [128, L], F32, kind="ExternalOutput")
        dbg_kt = nc.dram_tensor("dbg_kt", [128, L], F32, kind="ExternalOutput")
        dbg_vt = nc.dram_tensor("dbg_vt", [128, L], F32, kind="ExternalOutput")
        dbg_q2 = nc.dram_tensor("dbg_q2", [128, G], F32, kind="ExternalOutput")
        dbg_acc = nc.dram_tensor("dbg_acc", [128, L], F32, kind="ExternalOutput")
        dbg_zz = nc.dram_tensor("dbg_zz", [65, L], F32, kind="ExternalOutput")

    a2a_warm_in = nc.dram_tensor("a2a_warm_in", [8, 1, 64], F32R)
    a2a_warm_out = nc.dram_tensor("a2a_warm_out", [8, 1, 64], F32R)
    a2a_in = [nc.dram_tensor(f"a2a_in{h}", [8, 128, PB], F32R)
              for h in range(4)]
    a2a_out = [nc.dram_tensor(f"a2a_out{h}", [8, 128, PB], F32R)
               for h in range(4)]

    import contextlib

    def _emit(tc, ctx):
        pin = ctx.enter_context(tc.tile_pool(name="pin", bufs=2))
        persist = ctx.enter_context(tc.tile_pool(name="persist", bufs=1))
        vpool = ctx.enter_context(tc.tile_pool(name="vpool", bufs=2))
        epool = ctx.enter_context(tc.tile_pool(name="epool", bufs=4))
        opool = ctx.enter_context(tc.tile_pool(name="opool", bufs=1))
        psw = ctx.enter_context(tc.tile_pool(name="psw", bufs=2, space="PSUM"))
        pso = ctx.enter_context(tc.tile_pool(name="pso", bufs=2, space="PSUM"))

        # ---- small constants ----
        wq_sb = persist.tile([128, KC * 128], F32R, tag="wq")
        wk_sb = persist.tile([128, KC * 128], F32R, tag="wk")
        wv_sb = persist.tile([128, KC * 128], F32R, tag="wv")
        for w_d, w_t in ((wq, wq_sb), (wk, wk_sb), (wv, wv_sb)):
            wr = w_d.rearrange("(kc p) f -> kc p f", p=128)
            for kc in range(KC):
                nc.sync.dma_start(w_t[:, kc * 128:(kc + 1) * 128], wr[kc])
        bq_sb = persist.tile([128, 1], F32, tag="bq")
        bk_sb = persist.tile([128, 1], F32, tag="bk")
        bo_sb = persist.tile([128, 8], F32, tag="bo")
        ind_sb = persist.tile([2, 128], F32R, tag="ind")
        eye_sb = persist.tile([128, 128], F32R, tag="eye")
        on_sb = persist.tile([128, 16], F32R, tag="on")
        ws_sb = persist.tile([128, 2], F32, tag="ws")
        nc.sync.dma_start(bq_sb[:], bq[:])
        nc.sync.dma_start(bk_sb[:], bk[:])
        nc.sync.dma_start(bo_sb[:], bo8[:])
        nc.sync.dma_start(ind_sb[:], ind2[:])
        nc.sync.dma_start(eye_sb[:], eyer[:])
        nc.sync.dma_start(on_sb[:], ones16[:])
        nc.sync.dma_start(ws_sb[:], wsel[:])

        QT = persist.tile([128, L], F32R, tag="QT")
        KT = persist.tile([128, L], F32R, tag="KT")
        VT = persist.tile([128, L], F32R, tag="VT")
        QT2 = persist.tile([128, G], F32R, tag="QT2")
        KT2 = persist.tile([128, G], F32R, tag="KT2")
        VT2 = persist.tile([128, G], F32R, tag="VT2")
        acc = persist.tile([128, L], F32, tag="acc")
        zz = persist.tile([65, L], F32, tag="zz")

        streams = (
            ("k", kT, wk_sb, KT, bk_sb),
            ("v", vT, wv_sb, VT, None),
            ("q", qT, wq_sb, QT, bq_sb),
        )

        def proj_pb(pb):
            for name, x_d, w_t, dst, bias in streams:
                xin = pin.tile([128, KC * PB], F32R, tag="xin")
                xr = x_d.rearrange("(kc p) l -> kc p l", p=128)
                for kc in range(KC):
                    eng = (nc.sync, nc.gpsimd, nc.scalar)[kc % 3]
                    eng.dma_start(
                        xin[:, kc * PB:(kc + 1) * PB],
                        xr[kc][:, pb * PB:(pb + 1) * PB],
                    )
                pt = psw.tile([128, 1024], F32, tag="w")
                for kc in range(KC):
                    nc.tensor.matmul(
                        pt[:, 0:PB],
                        w_t[:, kc * 128:(kc + 1) * 128],
                        xin[:, kc * PB:(kc + 1) * PB],
                        start=(kc == 0), stop=(kc == KC - 1),
                    )
                dslice = dst[:, pb * PB:(pb + 1) * PB]
                if bias is not None:
                    nc.vector.tensor_scalar_add(dslice, pt[:, 0:PB], bias[:])
                else:
                    nc.vector.tensor_copy(dslice, pt[:, 0:PB])

        def b2_copies():
            # slot A picks dense offset 0 or 1, slot B picks 2 or 3, via
            # 0/1 indicators in ws_sb (core-uniform instruction stream).
            for src, dst in ((QT, QT2), (KT, KT2), (VT, VT2)):
                for slot in range(2):
                    p0, p1 = 64 * slot, 64 * slot + 64
                    o0 = 2 * slot
                    nc.vector.tensor_scalar_mul(
                        dst[p0:p1, :], src[p0:p1, o0::4], ws_sb[p0:p1, 0:1]
                    )
                    nc.vector.scalar_tensor_tensor(
                        dst[p0:p1, :], src[p0:p1, o0 + 1::4],
                        ws_sb[p0:p1, 1:2], dst[p0:p1, :],
                        mybir.AluOpType.mult, mybir.AluOpType.add,
                    )

        def kq_slice(br, seg, slot, t, lo, size):
            if br == 0:
                base = 1024 * seg + lo
                return t[slot * 64:(slot + 1) * 64, base:base + size]
            if br == 1:
                base = 2048 * seg + 2 * lo + slot
                return t[slot * 64:(slot + 1) * 64,
                         base:base + 2 * size - slot:2]
            return t[slot * 64:(slot + 1) * 64, lo:lo + size]

        def mm_ranges(jc):
            if jc <= 3:
                return [(128 * jc, 512 - 128 * jc), (512, 512)]
            return [(128 * jc, 1024 - 128 * jc)]

        def job(br, seg):
            kt_src = KT2 if br == 2 else KT
            qt_src = QT2 if br == 2 else QT
            # -- V_both prep --
            vb = vpool.tile([128, 2 * 8 * VBW], F32R, tag="vb")
            nc.vector.tensor_copy(vb[:, 64::VBW], on_sb[:])
            for jc in range(8):
                if br == 0:
                    tp = psw.tile([128, 1024], F32R, tag="w")
                    src = VT[:, 1024 * seg + 128 * jc:1024 * seg + 128 * (jc + 1)]
                    nc.tensor.transpose(tp[:, 0:128], src, eye_sb[:])
                    dst = vb[:].rearrange(
                        "p (s jj t) -> p s jj t", s=2, jj=8
                    )[:, :, jc, 0:64]
                    srcp = tp[:, 0:128].rearrange("p (s r) -> p s r", s=2)
                    nc.vector.tensor_copy(dst, srcp)
                else:
                    for slot in range(2):
                        tp = psw.tile([128, 1024], F32R, tag="w")
                        if br == 1:
                            base = 2048 * seg + 256 * jc + slot
                            src = VT[slot * 64:(slot + 1) * 64,
                                     base:base + 256 - slot:2]
                        else:
                            src = VT2[slot * 64:(slot + 1) * 64,
                                      128 * jc:128 * (jc + 1)]
                        nc.tensor.transpose(
                            tp[:, 0:64], src,
                            eye_sb[slot * 64:(slot + 1) * 64,
                                   slot * 64:(slot + 1) * 64],
                        )
                        nc.vector.tensor_copy(
                            vb[:, slot * 8 * VBW + jc * VBW:
                               slot * 8 * VBW + jc * VBW + 64],
                            tp[:, 0:64],
                        )

            o_ps_a = pso.tile([128, 1024], F32, tag="o")
            o_ps_b = pso.tile([128, 1024], F32, tag="o")
            o_ps = [o_ps_a, o_ps_b]

            for jc in range(8):
                s_ps_a = psw.tile([128, 1024], F32, tag="w")
                s_ps_b = psw.tile([128, 1024], F32, tag="w")
                s_ps = [s_ps_a, s_ps_b]
                for slot in range(2):
                    for lo, size in mm_ranges(jc):
                        lhs = kq_slice(br, seg, slot, kt_src, 128 * jc, 128)
                        rhs = kq_slice(br, seg, slot, qt_src, lo, size)
                        nc.tensor.matmul(
                            s_ps[slot][:, lo:lo + size], lhs, rhs,
                            start=True, stop=True,
                            tile_position=(slot * 64, 0),
                        )
                e_t = [None, None]
                for slot in range(2):
                    c0 = 128 * jc
                    e = epool.tile([128, 1024], F32R, tag="e")  # noqa
                    nc.scalar.activation(
                        e[:, c0:1024], s_ps[slot][:, c0:1024], AF.Exp
                    )
                    nc.gpsimd.affine_select(
                        e[:, c0:c0 + 128], e[:, c0:c0 + 128],
                        pattern=[[1, 128]],
                        compare_op=mybir.AluOpType.is_ge,
                        fill=0.0, base=0, channel_multiplier=-1,
                    )
                    e_t[slot] = e
                for slot in range(2):
                    for lo, size in mm_ranges(jc):
                        nc.tensor.matmul(
                            o_ps[slot][0:VBW, lo:lo + size],
                            vb[:, slot * 8 * VBW + jc * VBW:
                               slot * 8 * VBW + (jc + 1) * VBW],
                            e_t[slot][:, lo:lo + size],
                            start=(jc == 0), stop=(jc == 7),
                        )

            # -- merge into acc / zz (slot B copies on ACT to offload DVE) --
            for slot in range(2):
                op = o_ps[slot]
                po = slot * 64
                zr = 64 * slot
                if br == 0:
                    sl_ = slice(1024 * seg, 1024 * (seg + 1))
                    if slot == 0:
                        nc.vector.tensor_copy(acc[po:po + 64, sl_], op[0:64, :])
                        nc.vector.tensor_copy(zz[zr:zr + 1, sl_], op[64:65, :])
                    else:
                        nc.scalar.copy(acc[po:po + 64, sl_], op[0:64, :])
                        nc.scalar.copy(zz[zr:zr + 1, sl_], op[64:65, :])
                elif br == 1:
                    ac = acc[po:po + 64, 2048 * seg + slot:2048 * (seg + 1):2]
                    nc.vector.tensor_add(ac, ac, op[0:64, :])
                    zc = zz[zr:zr + 1, 2048 * seg + slot:2048 * (seg + 1):2]
                    nc.vector.tensor_add(zc, zc, op[64:65, :])
                else:
                    o0 = 2 * slot
                    for dd in range(2):
                        ac = acc[po:po + 64, o0 + dd::4]
                        nc.vector.scalar_tensor_tensor(
                            ac, op[0:64, :], ws_sb[po:po + 64, dd:dd + 1],
                            ac, mybir.AluOpType.mult, mybir.AluOpType.add,
                        )
                        zc = zz[zr:zr + 1, o0 + dd::4]
                        nc.vector.scalar_tensor_tensor(
                            zc, op[64:65, :], ws_sb[zr:zr + 1, dd:dd + 1],
                            zc, mybir.AluOpType.mult, mybir.AluOpType.add,
                        )

        # ================= emission order =================
        # tiny dummy collective first: absorbs the ~30us first-collective
        # setup cost while the projections stream.
        if stage >= 6:
            for rr in range(8):
                nc.sync.dma_start(a2a_warm_in[rr][0:1, 0:16], on_sb[0:1, 0:16])
            nc.gpsimd.collective_compute(
                "AllToAll", mybir.AluOpType.bypass,
                replica_groups=[list(range(8))],
                ins=[a2a_warm_in[:]], outs=[a2a_warm_out[:]],
            )
        proj_pb(0)
        proj_pb(1)
        if dbg and stage <= 1:
            for pbx in range(2, NPB):
                proj_pb(pbx)
            nc.sync.dma_start(dbg_qt[:], QT[:].bitcast(F32))
            nc.sync.dma_start(dbg_kt[:], KT[:].bitcast(F32))
            nc.sync.dma_start(dbg_vt[:], VT[:].bitcast(F32))
            return
        if stage >= 3:
            job(0, 0)
        proj_pb(2)
        proj_pb(3)
        if stage >= 4:
            job(0, 1)
            job(1, 0)
        proj_pb(4)
        proj_pb(5)
        if stage >= 4:
            job(0, 2)
        proj_pb(6)
        proj_pb(7)
        if stage >= 4:
            job(0, 3)
        b2_copies()
        if stage >= 4:
            job(2, 0)
            job(1, 1)

        if dbg:
            nc.sync.dma_start(dbg_qt[:], QT[:].bitcast(F32))
            nc.sync.dma_start(dbg_kt[:], KT[:].bitcast(F32))
            nc.sync.dma_start(dbg_vt[:], VT[:].bitcast(F32))
            nc.sync.dma_start(dbg_q2[:], QT2[:].bitcast(F32))
            if stage >= 3:
                nc.sync.dma_start(dbg_acc[:], acc[:])
                nc.sync.dma_start(dbg_zz[:], zz[:])
        if stage <= 4:
            return

        # ---- normalization (reciprocal reshaped to use all 128 lanes) ----
        zw = persist.tile([128, 64], F32R, tag="zw")
        for i, zr in enumerate((0, 64)):
            nc.sync.dma_start(
                zw[:, 32 * i:32 * i + 32].bitcast(F32), zz[zr:zr + 1, :]
            )
        with nc.allow_low_precision(reason="tf32 norm"):
            nc.vector.reciprocal(zw[:], zw[:])
        for pb in range(NPB):
            rzp = opool.tile([2, PB], F32R, tag="rzp")
            nc.sync.dma_start(rzp[0:1, :], zw[16 * pb:16 * pb + 16, 0:32])
            nc.sync.dma_start(rzp[1:2, :], zw[16 * pb:16 * pb + 16, 32:64])
            rb = psw.tile([128, 1024], F32, tag="w")
            nc.tensor.matmul(
                rb[:, 0:PB], ind_sb[:], rzp[:], start=True, stop=True,
            )
            aslice = acc[:, pb * PB:(pb + 1) * PB]
            nc.vector.tensor_mul(aslice, aslice, rb[:, 0:PB])
        if dbg and stage == 5:
            nc.sync.dma_start(dbg_acc[:], acc[:])
        if stage <= 5:
            return

        # ---- chunked AllToAll + output projection pipeline ----
        # split the position block into NCH chunks; a2a chunk h then overlaps
        # the output projection of chunk h-1.
        wo_sb_0 = pin.tile([128, 4 * E], F32R, tag="xin")
        wo_sb_1 = pin.tile([128, 4 * E], F32R, tag="xin")
        wo_sb = [wo_sb_0, wo_sb_1]
        for g in range(2):
            nc.sync.dma_start(wo_sb[g][:], wo[g])
        NCH = 1
        CW = PB // NCH  # chunk width within each 512 block
        mg = persist.tile([128, 8 * PB], F32R, tag="mg")
        for h in range(NCH):
            for r in range(8):
                nc.sync.dma_start(
                    a2a_in[h][r],
                    acc[:, PB * r + CW * h:PB * r + CW * (h + 1)].bitcast(F32R),
                )
            nc.gpsimd.collective_compute(
                "AllToAll", mybir.AluOpType.bypass,
                replica_groups=[list(range(8))],
                ins=[a2a_in[h][:]], outs=[a2a_out[h][:]],
            )
            for s in range(8):
                nc.sync.dma_start(
                    mg[:, s * PB + CW * h:s * PB + CW * (h + 1)], a2a_out[h][s]
                )
            for ob in range(8):
                pt = psw.tile([128, 1024], F32, tag="w")
                for ec in range(KC):
                    w_t = wo_sb[ec // 4]
                    lhs = w_t[:, (ec % 4) * E + ob * 128:
                              (ec % 4) * E + (ob + 1) * 128]
                    nc.tensor.matmul(
                        pt[:, 0:CW], lhs,
                        mg[:, ec * PB + CW * h:ec * PB + CW * (h + 1)],
                        start=(ec == 0), stop=(ec == KC - 1),
                    )
                osb = opool.tile([128, CW], F32, tag="osb")
                nc.vector.tensor_scalar_add(
                    osb[:], pt[:, 0:CW], bo_sb[:, ob:ob + 1]
                )
                nc.sync.dma_start(
                    outT[ob * 128:(ob + 1) * 128, CW * h:CW * (h + 1)], osb[:]
                )

    with tile.TileContext(nc) as tc, contextlib.ExitStack() as ctx:
        _emit(tc, ctx)

    nc.compile()
    return nc


_NC_CACHE = {}


def _get_nc(stage=6, dbg=False):
    key = (stage, dbg)
    if key not in _NC_CACHE:
        _NC_CACHE[key] = _build(stage, dbg)
    return _NC_CACHE[key]


def _prep_inputs(query, key, value, Wq, bq, Wk, bk, Wv, bv, Wo, bo):
    """Host-side sharding/layout prep. Returns in_maps for the 8 cores."""
    qT = np.ascontiguousarray(query[0].T)  # (E, L)
    kT = np.ascontiguousarray(key[0].T)
    vT = np.ascontiguousarray(value[0].T)

    WqT = np.ascontiguousarray(Wq.T) * np.float32(0.125)
    WkT = np.ascontiguousarray(Wk.T)
    WvT = np.ascontiguousarray(Wv.T)

    # permuted Wo.T rows to match a2a feature order
    perm = np.concatenate(
        [np.r_[64 * s:64 * s + 64, 512 + 64 * s:512 + 64 * s + 64]
         for s in range(8)]
    )
    WoT = np.ascontiguousarray(Wo.T)[perm]  # (E e', E o)
    wo_pack = np.zeros((2, 128, 4 * E), np.float32)
    for ec in range(8):
        wo_pack[ec // 4, :, (ec % 4) * E:(ec % 4 + 1) * E] = \
            WoT[ec * 128:(ec + 1) * 128]

    bo_eff = (bo + bv @ Wo.T).astype(np.float32)
    bo8 = bo_eff.reshape(8, 128).T.copy()  # [p, ob]

    # per-core offset indicators: slot A offset = c//4 in {0,1} on rows 0-63,
    # slot B offset = 2 + c//4 (encoded as its low bit) on rows 64-127.
    WS = np.zeros((8, 128, 2), np.float32)
    for c in range(8):
        d = c // 4
        WS[c, 0:64, d] = 1.0
        WS[c, 64:128, d] = 1.0

    IND = np.zeros((2, 128), np.float32)
    IND[0, 0:64] = 1.0
    IND[1, 64:128] = 1.0
    EYE = np.eye(128, dtype=np.float32)
    ONES16 = np.ones((128, 16), np.float32)

    in_maps = []
    for c in range(8):
        fa = np.r_[64 * c:64 * c + 64]
        fb = np.r_[512 + 64 * c:512 + 64 * c + 64]
        sel = np.concatenate([fa, fb])
        in_maps.append({
            "qT": qT, "kT": kT, "vT": vT,
            "wq": np.ascontiguousarray(WqT[:, sel]),
            "wk": np.ascontiguousarray(WkT[:, sel]),
            "wv": np.ascontiguousarray(WvT[:, sel]),
            "wo": wo_pack,
            "bq": (bq[sel] * np.float32(0.125)).reshape(128, 1).astype(np.float32),
            "bk": bk[sel].reshape(128, 1).astype(np.float32),
            "bo8": bo8,
            "ind2": IND, "eyer": EYE, "ones16": ONES16,
            "wsel": WS[c],
        })
    return in_maps


def kernel(query, key, value, Wq, bq, Wk, bk, Wv, bv, Wo, bo,
           _trace=False, _result_holder=None, _stage=6, _dbg=False):
    args = [np.asarray(a, np.float32) for a in
            (query, key, value, Wq, bq, Wk, bk, Wv, bv, Wo, bo)]
    nc = _get_nc(_stage, _dbg)
    in_maps = _prep_inputs(*args)
    res = bass_utils.run_bass_kernel_spmd(
        nc, in_maps, core_ids=list(range(N_CORES)), trace=_trace
    )
    if _result_holder is not None:
        _result_holder.append(res)
    outT = np.zeros((E, L), np.float32)
    for c in range(N_CORES):
        outT[:, PB * c:PB * (c + 1)] = res.results[c]["outT"]
    return np.ascontiguousarray(outT.T).reshape(1, L, E)


# revision 29
# speedup vs baseline: 1.1687x; 1.0305x over previous
"""Dilated (LongNet-style) attention kernel for 8 TRN2 NeuronCores.

Strategy (head-sharded, single AllToAll):
  - Core c owns heads {c, 8+c} (slot A / slot B). Slot A heads have branch-1
    dilation offset 0, slot B offset 1 (core-uniform), so the SPMD program is
    identical across cores; branch-2 offsets (c//4, 2+c//4) are selected via
    0/1 indicator scalars supplied as data.
  - Each core computes Q^T/K^T/V^T for its 128-feature slice from the FULL
    (host-transposed) inputs, runs all 14 of its segment attentions fully
    on-chip (scores transposed: [key, query] layout so the softmax needs no
    cross-partition reduction; Z row-sums come from an appended ones column
    in V), merges the three branches by scatter-accumulation, normalizes,
    then AllToAlls the merged head-block activations so each core can run
    the output projection for its own 512-position block.
  - All matmuls run in fp32r (TF32) at full PE rate. Jobs are emitted
    interleaved with the projection position-blocks they depend on so PE/ACT
    work overlaps the input streaming.
"""

import sys

if "/opt/trn_rl_repo" not in sys.path:
    sys.path.insert(0, "/opt/trn_rl_repo")

import numpy as np

import concourse.bacc as bacc
import concourse.bass as bass
import concourse.mybir as mybir
import concourse.tile as tile
from concourse import bass_utils

F32 = mybir.dt.float32
F32R = mybir.dt.float32r
AF = mybir.ActivationFunctionType

N_CORES = 8
E, L, H, D = 1024, 4096, 16, 64
KC = 8          # contraction chunks of 128 for the projections
PB = 512        # position block
NPB = L // PB   # 8
G = 1024        # compressed segment length (same for every branch)
VBW = 65        # V_both per-chunk width (64 feats + ones col)
NCH = 1         # a2a/outproj pipeline chunks

JOBS = [(0, 0), (0, 1), (1, 0), (0, 2), (0, 3), (1, 1), (2, 0)]


def _build(stage=6, dbg=False):
    """stage: 1=proj only, 2=+b2 copies, 3=+job b0s0, 4=+all jobs,
    5=+normalization, 6=full (a2a+outproj). dbg adds intermediate outputs."""
    nc = bacc.Bacc("TRN2", target_bir_lowering=False, debug=False,
                   num_devices=N_CORES)

    qT = nc.dram_tensor("qT", [E, L], F32R, kind="ExternalInput")
    kT = nc.dram_tensor("kT", [E, L], F32R, kind="ExternalInput")
    vT = nc.dram_tensor("vT", [E, L], F32R, kind="ExternalInput")
    wq = nc.dram_tensor("wq", [E, 128], F32R, kind="ExternalInput")
    wk = nc.dram_tensor("wk", [E, 128], F32R, kind="ExternalInput")
    wv = nc.dram_tensor("wv", [E, 128], F32R, kind="ExternalInput")
    wo = nc.dram_tensor("wo", [2, 128, 4 * E], F32R, kind="ExternalInput")
    bq = nc.dram_tensor("bq", [128, 1], F32, kind="ExternalInput")
    bk = nc.dram_tensor("bk", [128, 1], F32, kind="ExternalInput")
    bo8 = nc.dram_tensor("bo8", [128, 8], F32, kind="ExternalInput")
    ind2 = nc.dram_tensor("ind2", [2, 128], F32R, kind="ExternalInput")
    eyer = nc.dram_tensor("eyer", [128, 128], F32R, kind="ExternalInput")
    ones16 = nc.dram_tensor("ones16", [128, 16], F32R, kind="ExternalInput")
    wsel = nc.dram_tensor("wsel", [128, 2], F32, kind="ExternalInput")

    outT = nc.dram_tensor("outT", [E, PB], F32, kind="ExternalOutput")
    if dbg:
        dbg_qt = nc.dram_tensor("dbg_qt", [128, L], F32, kind="ExternalOutput")
        dbg_kt = nc.dram_tensor("dbg_kt", [128, L], F32, kind="ExternalOutput")
        dbg_vt = nc.dram_tensor("dbg_vt", [128, L], F32, kind="ExternalOutput")
        dbg_q2 = nc.dram_tensor("dbg_q2", [128, G], F32, kind="ExternalOutput")
        dbg_acc = nc.dram_tensor("dbg_acc", [128, L], F32, kind="ExternalOutput")
        dbg_zz = nc.dram_tensor("dbg_zz", [65, L], F32, kind="ExternalOutput")

    a2a_warm_in = nc.dram_tensor("a2a_warm_in", [8, 1, 64], F32R)
    a2a_warm_out = nc.dram_tensor("a2a_warm_out", [8, 1, 64], F32R)
    a2a_in = [nc.dram_tensor(f"a2a_in{h}", [8, 128, PB // NCH], F32R)
              for h in range(NCH)]
    a2a_out = [nc.dram_tensor(f"a2a_out{h}", [8, 128, PB // NCH], F32R)
               for h in range(NCH)]

    import contextlib

    def _emit(tc, ctx):
        pin = ctx.enter_context(tc.tile_pool(name="pin", bufs=3))
        persist = ctx.enter_context(tc.tile_pool(name="persist", bufs=1))
        vpool = ctx.enter_context(tc.tile_pool(name="vpool", bufs=2))
        epool = ctx.enter_context(tc.tile_pool(name="epool", bufs=4))
        opool = ctx.enter_context(tc.tile_pool(name="opool", bufs=1))
        psw = ctx.enter_context(tc.tile_pool(name="psw", bufs=2, space="PSUM"))
        pso = ctx.enter_context(tc.tile_pool(name="pso", bufs=2, space="PSUM"))

        # ---- small constants ----
        wq_sb = persist.tile([128, KC * 128], F32R, tag="wq")
        wk_sb = persist.tile([128, KC * 128], F32R, tag="wk")
        wv_sb = persist.tile([128, KC * 128], F32R, tag="wv")
        for w_d, w_t in ((wq, wq_sb), (wk, wk_sb), (wv, wv_sb)):
            wr = w_d.rearrange("(kc p) f -> kc p f", p=128)
            for kc in range(KC):
                nc.sync.dma_start(w_t[:, kc * 128:(kc + 1) * 128], wr[kc])
        bq_sb = persist.tile([128, 1], F32, tag="bq")
        bk_sb = persist.tile([128, 1], F32, tag="bk")
        bo_sb = persist.tile([128, 8], F32, tag="bo")
        ind_sb = persist.tile([2, 128], F32R, tag="ind")
        eye_sb = persist.tile([128, 128], F32R, tag="eye")
        on_sb = persist.tile([128, 16], F32R, tag="on")
        ws_sb = persist.tile([128, 2], F32, tag="ws")
        nc.sync.dma_start(bq_sb[:], bq[:])
        nc.sync.dma_start(bk_sb[:], bk[:])
        nc.sync.dma_start(bo_sb[:], bo8[:])
        nc.sync.dma_start(ind_sb[:], ind2[:])
        nc.sync.dma_start(eye_sb[:], eyer[:])
        nc.sync.dma_start(on_sb[:], ones16[:])
        nc.sync.dma_start(ws_sb[:], wsel[:])

        QT = persist.tile([128, L], F32R, tag="QT")
        KT = persist.tile([128, L], F32R, tag="KT")
        VT = persist.tile([128, L], F32R, tag="VT")
        QT2 = persist.tile([128, G], F32R, tag="QT2")
        KT2 = persist.tile([128, G], F32R, tag="KT2")
        VT2 = persist.tile([128, G], F32R, tag="VT2")
        acc = persist.tile([128, L], F32, tag="acc")
        zz = persist.tile([65, L], F32, tag="zz")

        streams = (
            ("k", kT, wk_sb, KT, bk_sb),
            ("v", vT, wv_sb, VT, None),
            ("q", qT, wq_sb, QT, bq_sb),
        )

        def proj_pb(pb):
            for name, x_d, w_t, dst, bias in streams:
                xin = pin.tile([128, KC * PB], F32R, tag="xin")
                xr = x_d.rearrange("(kc p) l -> kc p l", p=128)
                for kc in range(KC):
                    eng = (nc.sync, nc.gpsimd, nc.scalar)[kc % 3]
                    eng.dma_start(
                        xin[:, kc * PB:(kc + 1) * PB],
                        xr[kc][:, pb * PB:(pb + 1) * PB],
                    )
                pt = psw.tile([128, 1024], F32, tag="w")
                for kc in range(KC):
                    nc.tensor.matmul(
                        pt[:, 0:PB],
                        w_t[:, kc * 128:(kc + 1) * 128],
                        xin[:, kc * PB:(kc + 1) * PB],
                        start=(kc == 0), stop=(kc == KC - 1),
                    )
                dslice = dst[:, pb * PB:(pb + 1) * PB]
                if bias is not None:
                    nc.vector.tensor_scalar_add(dslice, pt[:, 0:PB], bias[:])
                else:
                    nc.vector.tensor_copy(dslice, pt[:, 0:PB])

        def b2_copies():
            # slot A picks dense offset 0 or 1, slot B picks 2 or 3, via
            # 0/1 indicators in ws_sb (core-uniform instruction stream).
            for src, dst in ((QT, QT2), (KT, KT2), (VT, VT2)):
                for slot in range(2):
                    p0, p1 = 64 * slot, 64 * slot + 64
                    o0 = 2 * slot
                    nc.vector.tensor_scalar_mul(
                        dst[p0:p1, :], src[p0:p1, o0::4], ws_sb[p0:p1, 0:1]
                    )
                    nc.vector.scalar_tensor_tensor(
                        dst[p0:p1, :], src[p0:p1, o0 + 1::4],
                        ws_sb[p0:p1, 1:2], dst[p0:p1, :],
                        mybir.AluOpType.mult, mybir.AluOpType.add,
                    )

        def kq_slice(br, seg, slot, t, lo, size):
            if br == 0:
                base = 1024 * seg + lo
                return t[slot * 64:(slot + 1) * 64, base:base + size]
            if br == 1:
                base = 2048 * seg + 2 * lo + slot
                return t[slot * 64:(slot + 1) * 64,
                         base:base + 2 * size - slot:2]
            return t[slot * 64:(slot + 1) * 64, lo:lo + size]

        def mm_ranges(jc):
            if jc <= 3:
                return [(128 * jc, 512 - 128 * jc), (512, 512)]
            return [(128 * jc, 1024 - 128 * jc)]

        def job(br, seg):
            kt_src = KT2 if br == 2 else KT
            qt_src = QT2 if br == 2 else QT
            # -- V_both prep --
            vb = vpool.tile([128, 2 * 8 * VBW], F32R, tag="vb")
            nc.vector.tensor_copy(vb[:, 64::VBW], on_sb[:])
            for jc in range(8):
                if br == 0:
                    tp = psw.tile([128, 1024], F32R, tag="w")
                    src = VT[:, 1024 * seg + 128 * jc:1024 * seg + 128 * (jc + 1)]
                    nc.tensor.transpose(tp[:, 0:128], src, eye_sb[:])
                    dst = vb[:].rearrange(
                        "p (s jj t) -> p s jj t", s=2, jj=8
                    )[:, :, jc, 0:64]
                    srcp = tp[:, 0:128].rearrange("p (s r) -> p s r", s=2)
                    nc.vector.tensor_copy(dst, srcp)
                else:
                    for slot in range(2):
                        tp = psw.tile([128, 1024], F32R, tag="w")
                        if br == 1:
                            base = 2048 * seg + 256 * jc + slot
                            src = VT[slot * 64:(slot + 1) * 64,
                                     base:base + 256 - slot:2]
                        else:
                            src = VT2[slot * 64:(slot + 1) * 64,
                                      128 * jc:128 * (jc + 1)]
                        nc.tensor.transpose(
                            tp[:, 0:64], src,
                            eye_sb[slot * 64:(slot + 1) * 64,
                                   slot * 64:(slot + 1) * 64],
                        )
                        nc.vector.tensor_copy(
                            vb[:, slot * 8 * VBW + jc * VBW:
                               slot * 8 * VBW + jc * VBW + 64],
                            tp[:, 0:64],
                        )

            o_ps_a = pso.tile([128, 1024], F32, tag="o")
            o_ps_b = pso.tile([128, 1024], F32, tag="o")
            o_ps = [o_ps_a, o_ps_b]

            for jc in range(8):
                s_ps_a = psw.tile([128, 1024], F32, tag="w")
                s_ps_b = psw.tile([128, 1024], F32, tag="w")
                s_ps = [s_ps_a, s_ps_b]
                for slot in range(2):
                    for lo, size in mm_ranges(jc):
                        lhs = kq_slice(br, seg, slot, kt_src, 128 * jc, 128)
                        rhs = kq_slice(br, seg, slot, qt_src, lo, size)
                        nc.tensor.matmul(
                            s_ps[slot][:, lo:lo + size], lhs, rhs,
                            start=True, stop=True,
                            tile_position=(slot * 64, 0),
                        )
                e_t = [None, None]
                for slot in range(2):
                    c0 = 128 * jc
                    e = epool.tile([128, 1024], F32R, tag="e")  # noqa
                    nc.scalar.activation(
                        e[:, c0:1024], s_ps[slot][:, c0:1024], AF.Exp
                    )
                    nc.gpsimd.affine_select(
                        e[:, c0:c0 + 128], e[:, c0:c0 + 128],
                        pattern=[[1, 128]],
                        compare_op=mybir.AluOpType.is_ge,
                        fill=0.0, base=0, channel_multiplier=-1,
                    )
                    e_t[slot] = e
                for slot in range(2):
                    for lo, size in mm_ranges(jc):
                        nc.tensor.matmul(
                            o_ps[slot][0:VBW, lo:lo + size],
                            vb[:, slot * 8 * VBW + jc * VBW:
                               slot * 8 * VBW + (jc + 1) * VBW],
                            e_t[slot][:, lo:lo + size],
                            start=(jc == 0), stop=(jc == 7),
                        )

            # -- merge into acc / zz (slot B copies on ACT to offload DVE) --
            for slot in range(2):
                op = o_ps[slot]
                po = slot * 64
                zr = 64 * slot
                if br == 0:
                    sl_ = slice(1024 * seg, 1024 * (seg + 1))
                    if slot == 0:
                        nc.vector.tensor_copy(acc[po:po + 64, sl_], op[0:64, :])
                        nc.vector.tensor_copy(zz[zr:zr + 1, sl_], op[64:65, :])
                    else:
                        nc.scalar.copy(acc[po:po + 64, sl_], op[0:64, :])
                        nc.scalar.copy(zz[zr:zr + 1, sl_], op[64:65, :])
                elif br == 1:
                    ac = acc[po:po + 64, 2048 * seg + slot:2048 * (seg + 1):2]
                    nc.vector.tensor_add(ac, ac, op[0:64, :])
                    zc = zz[zr:zr + 1, 2048 * seg + slot:2048 * (seg + 1):2]
                    nc.vector.tensor_add(zc, zc, op[64:65, :])
                else:
                    o0 = 2 * slot
                    for dd in range(2):
                        ac = acc[po:po + 64, o0 + dd::4]
                        nc.vector.scalar_tensor_tensor(
                            ac, op[0:64, :], ws_sb[po:po + 64, dd:dd + 1],
                            ac, mybir.AluOpType.mult, mybir.AluOpType.add,
                        )
                        zc = zz[zr:zr + 1, o0 + dd::4]
                        nc.vector.scalar_tensor_tensor(
                            zc, op[64:65, :], ws_sb[zr:zr + 1, dd:dd + 1],
                            zc, mybir.AluOpType.mult, mybir.AluOpType.add,
                        )

        # ================= emission order =================
        proj_pb(0)
        proj_pb(1)
        # tiny dummy collective: absorbs the ~30us first-collective setup
        # cost while the projections stream. Emitted after the first two
        # position blocks so their gpsimd-queue DMA chunks are not stalled
        # behind the collective trigger+wait.
        if stage >= 6:
            for rr in range(8):
                nc.sync.dma_start(a2a_warm_in[rr][0:1, 0:16], on_sb[0:1, 0:16])
            nc.gpsimd.collective_compute(
                "AllToAll", mybir.AluOpType.bypass,
                replica_groups=[list(range(8))],
                ins=[a2a_warm_in[:]], outs=[a2a_warm_out[:]],
            )
        if dbg and stage <= 1:
            for pbx in range(2, NPB):
                proj_pb(pbx)
            nc.sync.dma_start(dbg_qt[:], QT[:].bitcast(F32))
            nc.sync.dma_start(dbg_kt[:], KT[:].bitcast(F32))
            nc.sync.dma_start(dbg_vt[:], VT[:].bitcast(F32))
            return
        if stage >= 3:
            job(0, 0)
        proj_pb(2)
        proj_pb(3)
        if stage >= 4:
            job(0, 1)
            job(1, 0)
        proj_pb(4)
        proj_pb(5)
        if stage >= 4:
            job(0, 2)
        proj_pb(6)
        proj_pb(7)
        if stage >= 4:
            job(0, 3)
        b2_copies()
        wo_pre = []
        if stage >= 6:
            wo_sb_0 = pin.tile([128, 4 * E], F32R, tag="xin")
            wo_sb_1 = pin.tile([128, 4 * E], F32R, tag="xin")
            wo_pre = [wo_sb_0, wo_sb_1]
            for g in range(2):
                nc.sync.dma_start(wo_pre[g][:], wo[g])
        if stage >= 4:
            job(2, 0)
            job(1, 1)

        if dbg:
            nc.sync.dma_start(dbg_qt[:], QT[:].bitcast(F32))
            nc.sync.dma_start(dbg_kt[:], KT[:].bitcast(F32))
            nc.sync.dma_start(dbg_vt[:], VT[:].bitcast(F32))
            nc.sync.dma_start(dbg_q2[:], QT2[:].bitcast(F32))
            if stage >= 3:
                nc.sync.dma_start(dbg_acc[:], acc[:])
                nc.sync.dma_start(dbg_zz[:], zz[:])
        if stage <= 4:
            return

        # ---- normalization (reciprocal reshaped to use all 128 lanes) ----
        zw = persist.tile([128, 64], F32R, tag="zw")
        for i, zr in enumerate((0, 64)):
            nc.sync.dma_start(
                zw[:, 32 * i:32 * i + 32].bitcast(F32), zz[zr:zr + 1, :]
            )
        with nc.allow_low_precision(reason="tf32 norm"):
            nc.vector.reciprocal(zw[:], zw[:])
        for pb in range(NPB):
            rzp = opool.tile([2, PB], F32R, tag="rzp")
            nc.sync.dma_start(rzp[0:1, :], zw[16 * pb:16 * pb + 16, 0:32])
            nc.sync.dma_start(rzp[1:2, :], zw[16 * pb:16 * pb + 16, 32:64])
            rb = psw.tile([128, 1024], F32, tag="w")
            nc.tensor.matmul(
                rb[:, 0:PB], ind_sb[:], rzp[:], start=True, stop=True,
            )
            aslice = acc[:, pb * PB:(pb + 1) * PB]
            nc.vector.tensor_mul(aslice, aslice, rb[:, 0:PB])
            if stage >= 6 and NCH == 1:
                nc.sync.dma_start(
                    a2a_in[0][pb], acc[:, PB * pb:PB * (pb + 1)].bitcast(F32R)
                )
        if dbg and stage == 5:
            nc.sync.dma_start(dbg_acc[:], acc[:])
        if stage <= 5:
            return

        _staged = (NCH == 1)
        # ---- chunked AllToAll + output projection pipeline ----
        # split the position block into NCH chunks; a2a chunk h then overlaps
        # the output projection of chunk h-1.
        wo_sb = wo_pre
        CW = PB // NCH  # chunk width within each 512 block
        mg = persist.tile([128, 8 * PB], F32R, tag="acc")
        for h in range(NCH):
            if not _staged:
                for r in range(8):
                    nc.sync.dma_start(
                        a2a_in[h][r],
                        acc[:, PB * r + CW * h:PB * r + CW * (h + 1)].bitcast(F32R),
                    )
            nc.gpsimd.collective_compute(
                "AllToAll", mybir.AluOpType.bypass,
                replica_groups=[list(range(8))],
                ins=[a2a_in[h][:]], outs=[a2a_out[h][:]],
            )
            for s in range(8):
                nc.sync.dma_start(
                    mg[:, s * PB + CW * h:s * PB + CW * (h + 1)], a2a_out[h][s]
                )
            for ob in range(8):
                pt = psw.tile([128, 1024], F32, tag="w")
                for ec in range(KC):
                    w_t = wo_sb[ec // 4]
                    lhs = w_t[:, (ec % 4) * E + ob * 128:
                              (ec % 4) * E + (ob + 1) * 128]
                    nc.tensor.matmul(
                        pt[:, 0:CW], lhs,
                        mg[:, ec * PB + CW * h:ec * PB + CW * (h + 1)],
                        start=(ec == 0), stop=(ec == KC - 1),
                    )
                osb = opool.tile([128, CW], F32, tag="osb")
                nc.vector.tensor_scalar_add(
                    osb[:], pt[:, 0:CW], bo_sb[:, ob:ob + 1]
                )
                nc.sync.dma_start(
                    outT[ob * 128:(ob + 1) * 128, CW * h:CW * (h + 1)], osb[:]
                )

    with tile.TileContext(nc) as tc, contextlib.ExitStack() as ctx:
        _emit(tc, ctx)

    nc.compile()
    return nc


_NC_CACHE = {}


def _get_nc(stage=6, dbg=False):
    key = (stage, dbg)
    if key not in _NC_CACHE:
        _NC_CACHE[key] = _build(stage, dbg)
    return _NC_CACHE[key]


def _prep_inputs(query, key, value, Wq, bq, Wk, bk, Wv, bv, Wo, bo):
    """Host-side sharding/layout prep. Returns in_maps for the 8 cores."""
    qT = np.ascontiguousarray(query[0].T)  # (E, L)
    kT = np.ascontiguousarray(key[0].T)
    vT = np.ascontiguousarray(value[0].T)

    WqT = np.ascontiguousarray(Wq.T) * np.float32(0.125)
    WkT = np.ascontiguousarray(Wk.T)
    WvT = np.ascontiguousarray(Wv.T)

    # permuted Wo.T rows to match a2a feature order
    perm = np.concatenate(
        [np.r_[64 * s:64 * s + 64, 512 + 64 * s:512 + 64 * s + 64]
         for s in range(8)]
    )
    WoT = np.ascontiguousarray(Wo.T)[perm]  # (E e', E o)
    wo_pack = np.zeros((2, 128, 4 * E), np.float32)
    for ec in range(8):
        wo_pack[ec // 4, :, (ec % 4) * E:(ec % 4 + 1) * E] = \
            WoT[ec * 128:(ec + 1) * 128]

    bo_eff = (bo + bv @ Wo.T).astype(np.float32)
    bo8 = bo_eff.reshape(8, 128).T.copy()  # [p, ob]

    # per-core offset indicators: slot A offset = c//4 in {0,1} on rows 0-63,
    # slot B offset = 2 + c//4 (encoded as its low bit) on rows 64-127.
    WS = np.zeros((8, 128, 2), np.float32)
    for c in range(8):
        d = c // 4
        WS[c, 0:64, d] = 1.0
        WS[c, 64:128, d] = 1.0

    IND = np.zeros((2, 128), np.float32)
    IND[0, 0:64] = 1.0
    IND[1, 64:128] = 1.0
    EYE = np.eye(128, dtype=np.float32)
    ONES16 = np.ones((128, 16), np.float32)

    in_maps = []
    for c in range(8):
        fa = np.r_[64 * c:64 * c + 64]
        fb = np.r_[512 + 64 * c:512 + 64 * c + 64]
        sel = np.concatenate([fa, fb])
        in_maps.append({
            "qT": qT, "kT": kT, "vT": vT,
            "wq": np.ascontiguousarray(WqT[:, sel]),
            "wk": np.ascontiguousarray(WkT[:, sel]),
            "wv": np.ascontiguousarray(WvT[:, sel]),
            "wo": wo_pack,
            "bq": (bq[sel] * np.float32(0.125)).reshape(128, 1).astype(np.float32),
            "bk": bk[sel].reshape(128, 1).astype(np.float32),
            "bo8": bo8,
            "ind2": IND, "eyer": EYE, "ones16": ONES16,
            "wsel": WS[c],
        })
    return in_maps


def kernel(query, key, value, Wq, bq, Wk, bk, Wv, bv, Wo, bo,
           _trace=False, _result_holder=None, _stage=6, _dbg=False):
    args = [np.asarray(a, np.float32) for a in
            (query, key, value, Wq, bq, Wk, bk, Wv, bv, Wo, bo)]
    nc = _get_nc(_stage, _dbg)
    in_maps = _prep_inputs(*args)
    res = bass_utils.run_bass_kernel_spmd(
        nc, in_maps, core_ids=list(range(N_CORES)), trace=_trace
    )
    if _result_holder is not None:
        _result_holder.append(res)
    outT = np.zeros((E, L), np.float32)
    for c in range(N_CORES):
        outT[:, PB * c:PB * (c + 1)] = res.results[c]["outT"]
    return np.ascontiguousarray(outT.T).reshape(1, L, E)


# revision 30
# speedup vs baseline: 1.2367x; 1.0582x over previous
"""Dilated (LongNet-style) attention kernel for 8 TRN2 NeuronCores.

Strategy (head-sharded, single AllToAll):
  - Core c owns heads {c, 8+c} (slot A / slot B). Slot A heads have branch-1
    dilation offset 0, slot B offset 1 (core-uniform), so the SPMD program is
    identical across cores; branch-2 offsets (c//4, 2+c//4) are selected via
    0/1 indicator scalars supplied as data.
  - Each core computes Q^T/K^T/V^T for its 128-feature slice from the FULL
    (host-transposed) inputs, runs all 14 of its segment attentions fully
    on-chip (scores transposed: [key, query] layout so the softmax needs no
    cross-partition reduction; Z row-sums come from an appended ones column
    in V), merges the three branches by scatter-accumulation, normalizes,
    then AllToAlls the merged head-block activations so each core can run
    the output projection for its own 512-position block.
  - All matmuls run in fp32r (TF32) at full PE rate. Jobs are emitted
    interleaved with the projection position-blocks they depend on so PE/ACT
    work overlaps the input streaming.
"""

import sys

if "/opt/trn_rl_repo" not in sys.path:
    sys.path.insert(0, "/opt/trn_rl_repo")

import numpy as np

import concourse.bacc as bacc
import concourse.bass as bass
import concourse.mybir as mybir
import concourse.tile as tile
from concourse import bass_utils

F32 = mybir.dt.float32
F32R = mybir.dt.float32r
AF = mybir.ActivationFunctionType

N_CORES = 8
E, L, H, D = 1024, 4096, 16, 64
KC = 8          # contraction chunks of 128 for the projections
PB = 512        # position block
NPB = L // PB   # 8
G = 1024        # compressed segment length (same for every branch)
VBW = 65        # V_both per-chunk width (64 feats + ones col)
NCH = 1         # a2a/outproj pipeline chunks

JOBS = [(0, 0), (0, 1), (1, 0), (0, 2), (0, 3), (1, 1), (2, 0)]


def _build(stage=6, dbg=False):
    """stage: 1=proj only, 2=+b2 copies, 3=+job b0s0, 4=+all jobs,
    5=+normalization, 6=full (a2a+outproj). dbg adds intermediate outputs."""
    nc = bacc.Bacc("TRN2", target_bir_lowering=False, debug=False,
                   num_devices=N_CORES)

    qT = nc.dram_tensor("qT", [E, L], F32R, kind="ExternalInput")
    kT = nc.dram_tensor("kT", [E, L], F32R, kind="ExternalInput")
    vT = nc.dram_tensor("vT", [E, L], F32R, kind="ExternalInput")
    wq = nc.dram_tensor("wq", [E, 128], F32R, kind="ExternalInput")
    wk = nc.dram_tensor("wk", [E, 128], F32R, kind="ExternalInput")
    wv = nc.dram_tensor("wv", [E, 128], F32R, kind="ExternalInput")
    wo = nc.dram_tensor("wo", [2, 128, 4 * E], F32R, kind="ExternalInput")
    bq = nc.dram_tensor("bq", [128, 1], F32, kind="ExternalInput")
    bk = nc.dram_tensor("bk", [128, 1], F32, kind="ExternalInput")
    bo8 = nc.dram_tensor("bo8", [128, 8], F32, kind="ExternalInput")
    ind2 = nc.dram_tensor("ind2", [2, 128], F32R, kind="ExternalInput")
    eyer = nc.dram_tensor("eyer", [128, 128], F32R, kind="ExternalInput")
    ones16 = nc.dram_tensor("ones16", [128, 16], F32R, kind="ExternalInput")
    wsel = nc.dram_tensor("wsel", [128, 2], F32, kind="ExternalInput")

    outT = nc.dram_tensor("outT", [E, PB], F32, kind="ExternalOutput")
    if dbg:
        dbg_qt = nc.dram_tensor("dbg_qt", [128, L], F32, kind="ExternalOutput")
        dbg_kt = nc.dram_tensor("dbg_kt", [128, L], F32, kind="ExternalOutput")
        dbg_vt = nc.dram_tensor("dbg_vt", [128, L], F32, kind="ExternalOutput")
        dbg_q2 = nc.dram_tensor("dbg_q2", [128, G], F32, kind="ExternalOutput")
        dbg_acc = nc.dram_tensor("dbg_acc", [128, L], F32, kind="ExternalOutput")
        dbg_zz = nc.dram_tensor("dbg_zz", [65, L], F32, kind="ExternalOutput")

    a2a_warm_in = nc.dram_tensor("a2a_warm_in", [8, 1, 64], F32R)
    a2a_warm_out = nc.dram_tensor("a2a_warm_out", [8, 1, 64], F32R)
    a2a_in = [nc.dram_tensor(f"a2a_in{h}", [8, 128, PB // NCH], F32R)
              for h in range(NCH)]
    a2a_out = [nc.dram_tensor(f"a2a_out{h}", [8, 128, PB // NCH], F32R)
               for h in range(NCH)]

    import contextlib

    def _emit(tc, ctx):
        pin = ctx.enter_context(tc.tile_pool(name="pin", bufs=3))
        persist = ctx.enter_context(tc.tile_pool(name="persist", bufs=1))
        vpool = ctx.enter_context(tc.tile_pool(name="vpool", bufs=2))
        epool = ctx.enter_context(tc.tile_pool(name="epool", bufs=4))
        opool = ctx.enter_context(tc.tile_pool(name="opool", bufs=2))
        psw = ctx.enter_context(tc.tile_pool(name="psw", bufs=2, space="PSUM"))
        pso = ctx.enter_context(tc.tile_pool(name="pso", bufs=2, space="PSUM"))

        # ---- small constants ----
        wq_sb = persist.tile([128, KC * 128], F32R, tag="wq")
        wk_sb = persist.tile([128, KC * 128], F32R, tag="wk")
        wv_sb = persist.tile([128, KC * 128], F32R, tag="wv")
        for w_d, w_t in ((wq, wq_sb), (wk, wk_sb), (wv, wv_sb)):
            wr = w_d.rearrange("(kc p) f -> kc p f", p=128)
            for kc in range(KC):
                nc.sync.dma_start(w_t[:, kc * 128:(kc + 1) * 128], wr[kc])
        bq_sb = persist.tile([128, 1], F32, tag="bq")
        bk_sb = persist.tile([128, 1], F32, tag="bk")
        bo_sb = persist.tile([128, 8], F32, tag="bo")
        ind_sb = persist.tile([2, 128], F32R, tag="ind")
        eye_sb = persist.tile([128, 128], F32R, tag="eye")
        on_sb = persist.tile([128, 16], F32R, tag="on")
        ws_sb = persist.tile([128, 2], F32, tag="ws")
        nc.sync.dma_start(bq_sb[:], bq[:])
        nc.sync.dma_start(bk_sb[:], bk[:])
        nc.sync.dma_start(bo_sb[:], bo8[:])
        nc.sync.dma_start(ind_sb[:], ind2[:])
        nc.sync.dma_start(eye_sb[:], eyer[:])
        nc.sync.dma_start(on_sb[:], ones16[:])
        nc.sync.dma_start(ws_sb[:], wsel[:])

        QT = persist.tile([128, L], F32R, tag="QT")
        KT = persist.tile([128, L], F32R, tag="KT")
        VT = persist.tile([128, L], F32R, tag="VT")
        QT2 = persist.tile([128, G], F32R, tag="QT2")
        KT2 = persist.tile([128, G], F32R, tag="KT2")
        VT2 = persist.tile([128, G], F32R, tag="VT2")
        acc = persist.tile([128, L], F32, tag="acc")
        zz = persist.tile([65, L], F32, tag="zz")

        streams = (
            ("k", kT, wk_sb, KT, bk_sb),
            ("v", vT, wv_sb, VT, None),
            ("q", qT, wq_sb, QT, bq_sb),
        )

        def proj_pb(pb):
            for name, x_d, w_t, dst, bias in streams:
                xin = pin.tile([128, KC * PB], F32R, tag="xin")
                xr = x_d.rearrange("(kc p) l -> kc p l", p=128)
                for kc in range(KC):
                    eng = (nc.sync, nc.gpsimd, nc.scalar)[kc % 3]
                    eng.dma_start(
                        xin[:, kc * PB:(kc + 1) * PB],
                        xr[kc][:, pb * PB:(pb + 1) * PB],
                    )
                pt = psw.tile([128, 1024], F32, tag="w")
                for kc in range(KC):
                    nc.tensor.matmul(
                        pt[:, 0:PB],
                        w_t[:, kc * 128:(kc + 1) * 128],
                        xin[:, kc * PB:(kc + 1) * PB],
                        start=(kc == 0), stop=(kc == KC - 1),
                    )
                dslice = dst[:, pb * PB:(pb + 1) * PB]
                if bias is not None:
                    nc.vector.tensor_scalar_add(dslice, pt[:, 0:PB], bias[:])
                else:
                    nc.vector.tensor_copy(dslice, pt[:, 0:PB])

        def b2_copies():
            # slot A picks dense offset 0 or 1, slot B picks 2 or 3, via
            # 0/1 indicators in ws_sb (core-uniform instruction stream).
            for src, dst in ((QT, QT2), (KT, KT2), (VT, VT2)):
                for slot in range(2):
                    p0, p1 = 64 * slot, 64 * slot + 64
                    o0 = 2 * slot
                    nc.vector.tensor_scalar_mul(
                        dst[p0:p1, :], src[p0:p1, o0::4], ws_sb[p0:p1, 0:1]
                    )
                    nc.vector.scalar_tensor_tensor(
                        dst[p0:p1, :], src[p0:p1, o0 + 1::4],
                        ws_sb[p0:p1, 1:2], dst[p0:p1, :],
                        mybir.AluOpType.mult, mybir.AluOpType.add,
                    )

        def kq_slice(br, seg, slot, t, lo, size):
            if br == 0:
                base = 1024 * seg + lo
                return t[slot * 64:(slot + 1) * 64, base:base + size]
            if br == 1:
                base = 2048 * seg + 2 * lo + slot
                return t[slot * 64:(slot + 1) * 64,
                         base:base + 2 * size - slot:2]
            return t[slot * 64:(slot + 1) * 64, lo:lo + size]

        def mm_ranges(jc):
            if jc <= 3:
                return [(128 * jc, 512 - 128 * jc), (512, 512)]
            return [(128 * jc, 1024 - 128 * jc)]

        def job(br, seg):
            kt_src = KT2 if br == 2 else KT
            qt_src = QT2 if br == 2 else QT
            # -- V_both prep --
            vb = vpool.tile([128, 2 * 8 * VBW], F32R, tag="vb")
            nc.vector.tensor_copy(vb[:, 64::VBW], on_sb[:])
            for jc in range(8):
                if br == 0:
                    tp = psw.tile([128, 1024], F32R, tag="w")
                    src = VT[:, 1024 * seg + 128 * jc:1024 * seg + 128 * (jc + 1)]
                    nc.tensor.transpose(tp[:, 0:128], src, eye_sb[:])
                    dst = vb[:].rearrange(
                        "p (s jj t) -> p s jj t", s=2, jj=8
                    )[:, :, jc, 0:64]
                    srcp = tp[:, 0:128].rearrange("p (s r) -> p s r", s=2)
                    nc.vector.tensor_copy(dst, srcp)
                else:
                    for slot in range(2):
                        tp = psw.tile([128, 1024], F32R, tag="w")
                        if br == 1:
                            base = 2048 * seg + 256 * jc + slot
                            src = VT[slot * 64:(slot + 1) * 64,
                                     base:base + 256 - slot:2]
                        else:
                            src = VT2[slot * 64:(slot + 1) * 64,
                                      128 * jc:128 * (jc + 1)]
                        nc.tensor.transpose(
                            tp[:, 0:64], src,
                            eye_sb[slot * 64:(slot + 1) * 64,
                                   slot * 64:(slot + 1) * 64],
                        )
                        nc.vector.tensor_copy(
                            vb[:, slot * 8 * VBW + jc * VBW:
                               slot * 8 * VBW + jc * VBW + 64],
                            tp[:, 0:64],
                        )

            o_ps_a = pso.tile([128, 1024], F32, tag="o")
            o_ps_b = pso.tile([128, 1024], F32, tag="o")
            o_ps = [o_ps_a, o_ps_b]

            for jc in range(8):
                s_ps_a = psw.tile([128, 1024], F32, tag="w")
                s_ps_b = psw.tile([128, 1024], F32, tag="w")
                s_ps = [s_ps_a, s_ps_b]
                for slot in range(2):
                    for lo, size in mm_ranges(jc):
                        lhs = kq_slice(br, seg, slot, kt_src, 128 * jc, 128)
                        rhs = kq_slice(br, seg, slot, qt_src, lo, size)
                        nc.tensor.matmul(
                            s_ps[slot][:, lo:lo + size], lhs, rhs,
                            start=True, stop=True,
                            tile_position=(slot * 64, 0),
                        )
                e_t = [None, None]
                for slot in range(2):
                    c0 = 128 * jc
                    e = epool.tile([128, 1024], F32R, tag="e")  # noqa
                    nc.scalar.activation(
                        e[:, c0:1024], s_ps[slot][:, c0:1024], AF.Exp
                    )
                    nc.gpsimd.affine_select(
                        e[:, c0:c0 + 128], e[:, c0:c0 + 128],
                        pattern=[[1, 128]],
                        compare_op=mybir.AluOpType.is_ge,
                        fill=0.0, base=0, channel_multiplier=-1,
                    )
                    e_t[slot] = e
                for slot in range(2):
                    for lo, size in mm_ranges(jc):
                        nc.tensor.matmul(
                            o_ps[slot][0:VBW, lo:lo + size],
                            vb[:, slot * 8 * VBW + jc * VBW:
                               slot * 8 * VBW + (jc + 1) * VBW],
                            e_t[slot][:, lo:lo + size],
                            start=(jc == 0), stop=(jc == 7),
                        )

            # -- merge into acc / zz (slot B copies on ACT to offload DVE) --
            for slot in range(2):
                op = o_ps[slot]
                po = slot * 64
                zr = 64 * slot
                if br == 0:
                    sl_ = slice(1024 * seg, 1024 * (seg + 1))
                    if slot == 0:
                        nc.vector.tensor_copy(acc[po:po + 64, sl_], op[0:64, :])
                        nc.vector.tensor_copy(zz[zr:zr + 1, sl_], op[64:65, :])
                    else:
                        nc.scalar.copy(acc[po:po + 64, sl_], op[0:64, :])
                        nc.scalar.copy(zz[zr:zr + 1, sl_], op[64:65, :])
                elif br == 1:
                    ac = acc[po:po + 64, 2048 * seg + slot:2048 * (seg + 1):2]
                    nc.vector.tensor_add(ac, ac, op[0:64, :])
                    zc = zz[zr:zr + 1, 2048 * seg + slot:2048 * (seg + 1):2]
                    nc.vector.tensor_add(zc, zc, op[64:65, :])
                else:
                    o0 = 2 * slot
                    for dd in range(2):
                        ac = acc[po:po + 64, o0 + dd::4]
                        nc.vector.scalar_tensor_tensor(
                            ac, op[0:64, :], ws_sb[po:po + 64, dd:dd + 1],
                            ac, mybir.AluOpType.mult, mybir.AluOpType.add,
                        )
                        zc = zz[zr:zr + 1, o0 + dd::4]
                        nc.vector.scalar_tensor_tensor(
                            zc, op[64:65, :], ws_sb[zr:zr + 1, dd:dd + 1],
                            zc, mybir.AluOpType.mult, mybir.AluOpType.add,
                        )

        # ================= emission order =================
        proj_pb(0)
        proj_pb(1)
        # tiny dummy collective: absorbs the ~30us first-collective setup
        # cost while the projections stream. Emitted after the first two
        # position blocks so their gpsimd-queue DMA chunks are not stalled
        # behind the collective trigger+wait.
        if stage >= 6:
            for rr in range(8):
                nc.sync.dma_start(a2a_warm_in[rr][0:1, 0:16], on_sb[0:1, 0:16])
            nc.gpsimd.collective_compute(
                "AllToAll", mybir.AluOpType.bypass,
                replica_groups=[list(range(8))],
                ins=[a2a_warm_in[:]], outs=[a2a_warm_out[:]],
            )
        if dbg and stage <= 1:
            for pbx in range(2, NPB):
                proj_pb(pbx)
            nc.sync.dma_start(dbg_qt[:], QT[:].bitcast(F32))
            nc.sync.dma_start(dbg_kt[:], KT[:].bitcast(F32))
            nc.sync.dma_start(dbg_vt[:], VT[:].bitcast(F32))
            return
        if stage >= 3:
            job(0, 0)
        proj_pb(2)
        proj_pb(3)
        if stage >= 4:
            job(0, 1)
            job(1, 0)
        proj_pb(4)
        proj_pb(5)
        if stage >= 4:
            job(0, 2)
        proj_pb(6)
        proj_pb(7)
        if stage >= 4:
            job(0, 3)
        b2_copies()
        wo_pre = []
        if stage >= 6:
            wo_sb_0 = pin.tile([128, 4 * E], F32R, tag="xin")
            wo_sb_1 = pin.tile([128, 4 * E], F32R, tag="xin")
            wo_pre = [wo_sb_0, wo_sb_1]
            for g in range(2):
                nc.sync.dma_start(wo_pre[g][:], wo[g])
        if stage >= 4:
            job(2, 0)
            job(1, 1)

        if dbg:
            nc.sync.dma_start(dbg_qt[:], QT[:].bitcast(F32))
            nc.sync.dma_start(dbg_kt[:], KT[:].bitcast(F32))
            nc.sync.dma_start(dbg_vt[:], VT[:].bitcast(F32))
            nc.sync.dma_start(dbg_q2[:], QT2[:].bitcast(F32))
            if stage >= 3:
                nc.sync.dma_start(dbg_acc[:], acc[:])
                nc.sync.dma_start(dbg_zz[:], zz[:])
        if stage <= 4:
            return

        # ---- normalization (reciprocal reshaped to use all 128 lanes) ----
        zw = persist.tile([128, 64], F32R, tag="zw")
        for i, zr in enumerate((0, 64)):
            nc.sync.dma_start(
                zw[:, 32 * i:32 * i + 32].bitcast(F32), zz[zr:zr + 1, :]
            )
        with nc.allow_low_precision(reason="tf32 norm"):
            nc.vector.reciprocal(zw[:], zw[:])
        for pb in range(NPB):
            rzp = opool.tile([2, PB], F32R, tag="rzp")
            nc.sync.dma_start(rzp[0:1, :], zw[16 * pb:16 * pb + 16, 0:32])
            nc.sync.dma_start(rzp[1:2, :], zw[16 * pb:16 * pb + 16, 32:64])
            rb = psw.tile([128, 1024], F32, tag="w")
            nc.tensor.matmul(
                rb[:, 0:PB], ind_sb[:], rzp[:], start=True, stop=True,
            )
            aslice = acc[:, pb * PB:(pb + 1) * PB]
            nc.vector.tensor_mul(aslice, aslice, rb[:, 0:PB])
            if stage >= 6 and NCH == 1:
                nc.sync.dma_start(
                    a2a_in[0][pb], acc[:, PB * pb:PB * (pb + 1)].bitcast(F32R)
                )
        if dbg and stage == 5:
            nc.sync.dma_start(dbg_acc[:], acc[:])
        if stage <= 5:
            return

        _staged = (NCH == 1)
        # ---- chunked AllToAll + output projection pipeline ----
        # split the position block into NCH chunks; a2a chunk h then overlaps
        # the output projection of chunk h-1.
        wo_sb = wo_pre
        CW = PB // NCH  # chunk width within each 512 block
        mg = persist.tile([128, 8 * PB], F32R, tag="acc")
        for h in range(NCH):
            if not _staged:
                for r in range(8):
                    nc.sync.dma_start(
                        a2a_in[h][r],
                        acc[:, PB * r + CW * h:PB * r + CW * (h + 1)].bitcast(F32R),
                    )
            nc.gpsimd.collective_compute(
                "AllToAll", mybir.AluOpType.bypass,
                replica_groups=[list(range(8))],
                ins=[a2a_in[h][:]], outs=[a2a_out[h][:]],
            )
            for s in range(8):
                nc.sync.dma_start(
                    mg[:, s * PB + CW * h:s * PB + CW * (h + 1)], a2a_out[h][s]
                )
            for ob in range(8):
                pt = psw.tile([128, 1024], F32, tag="w")
                for ec in range(KC):
                    w_t = wo_sb[ec // 4]
                    lhs = w_t[:, (ec % 4) * E + ob * 128:
                              (ec % 4) * E + (ob + 1) * 128]
                    nc.tensor.matmul(
                        pt[:, 0:CW], lhs,
                        mg[:, ec * PB + CW * h:ec * PB + CW * (h + 1)],
                        start=(ec == 0), stop=(ec == KC - 1),
                    )
                osb = opool.tile([128, CW], F32, tag="osb")
                nc.vector.tensor_scalar_add(
                    osb[:], pt[:, 0:CW], bo_sb[:, ob:ob + 1]
                )
                nc.sync.dma_start(
                    outT[ob * 128:(ob + 1) * 128, CW * h:CW * (h + 1)], osb[:]
                )

    with tile.TileContext(nc) as tc, contextlib.ExitStack() as ctx:
        _emit(tc, ctx)

    nc.compile()
    return nc


_NC_CACHE = {}


def _get_nc(stage=6, dbg=False):
    key = (stage, dbg)
    if key not in _NC_CACHE:
        _NC_CACHE[key] = _build(stage, dbg)
    return _NC_CACHE[key]


def _prep_inputs(query, key, value, Wq, bq, Wk, bk, Wv, bv, Wo, bo):
    """Host-side sharding/layout prep. Returns in_maps for the 8 cores."""
    qT = np.ascontiguousarray(query[0].T)  # (E, L)
    kT = np.ascontiguousarray(key[0].T)
    vT = np.ascontiguousarray(value[0].T)

    WqT = np.ascontiguousarray(Wq.T) * np.float32(0.125)
    WkT = np.ascontiguousarray(Wk.T)
    WvT = np.ascontiguousarray(Wv.T)

    # permuted Wo.T rows to match a2a feature order
    perm = np.concatenate(
        [np.r_[64 * s:64 * s + 64, 512 + 64 * s:512 + 64 * s + 64]
         for s in range(8)]
    )
    WoT = np.ascontiguousarray(Wo.T)[perm]  # (E e', E o)
    wo_pack = np.zeros((2, 128, 4 * E), np.float32)
    for ec in range(8):
        wo_pack[ec // 4, :, (ec % 4) * E:(ec % 4 + 1) * E] = \
            WoT[ec * 128:(ec + 1) * 128]

    bo_eff = (bo + bv @ Wo.T).astype(np.float32)
    bo8 = bo_eff.reshape(8, 128).T.copy()  # [p, ob]

    # per-core offset indicators: slot A offset = c//4 in {0,1} on rows 0-63,
    # slot B offset = 2 + c//4 (encoded as its low bit) on rows 64-127.
    WS = np.zeros((8, 128, 2), np.float32)
    for c in range(8):
        d = c // 4
        WS[c, 0:64, d] = 1.0
        WS[c, 64:128, d] = 1.0

    IND = np.zeros((2, 128), np.float32)
    IND[0, 0:64] = 1.0
    IND[1, 64:128] = 1.0
    EYE = np.eye(128, dtype=np.float32)
    ONES16 = np.ones((128, 16), np.float32)

    in_maps = []
    for c in range(8):
        fa = np.r_[64 * c:64 * c + 64]
        fb = np.r_[512 + 64 * c:512 + 64 * c + 64]
        sel = np.concatenate([fa, fb])
        in_maps.append({
            "qT": qT, "kT": kT, "vT": vT,
            "wq": np.ascontiguousarray(WqT[:, sel]),
            "wk": np.ascontiguousarray(WkT[:, sel]),
            "wv": np.ascontiguousarray(WvT[:, sel]),
            "wo": wo_pack,
            "bq": (bq[sel] * np.float32(0.125)).reshape(128, 1).astype(np.float32),
            "bk": bk[sel].reshape(128, 1).astype(np.float32),
            "bo8": bo8,
            "ind2": IND, "eyer": EYE, "ones16": ONES16,
            "wsel": WS[c],
        })
    return in_maps


def kernel(query, key, value, Wq, bq, Wk, bk, Wv, bv, Wo, bo,
           _trace=False, _result_holder=None, _stage=6, _dbg=False):
    args = [np.asarray(a, np.float32) for a in
            (query, key, value, Wq, bq, Wk, bk, Wv, bv, Wo, bo)]
    nc = _get_nc(_stage, _dbg)
    in_maps = _prep_inputs(*args)
    res = bass_utils.run_bass_kernel_spmd(
        nc, in_maps, core_ids=list(range(N_CORES)), trace=_trace
    )
    if _result_holder is not None:
        _result_holder.append(res)
    outT = np.zeros((E, L), np.float32)
    for c in range(N_CORES):
        outT[:, PB * c:PB * (c + 1)] = res.results[c]["outT"]
    return np.ascontiguousarray(outT.T).reshape(1, L, E)
